# revision 5
# baseline (speedup 1.0000x reference)
"""Trainium2 Bass kernel: 3x3 conv2d (stride 1, pad 1), NCHW.

x (32, 64, 112, 112) f32, weight (1, 128, 64, 3, 3) f32 -> out (32, 128, 112, 112) f32.

Strategy: data-parallel over batch across 8 cores (4 images/core).
Per core, conv is computed as PSUM-accumulating matmuls over kernel taps:
x is host-padded to (114, 114) so each tap's shifted input window is a
constant free-dim offset into the flat [114*114] SBUF image. Output is
produced in padded row-major (112 x 114) layout and sliced on the host.

Default variant "dr4": fp8(e4m3) DoubleRow matmuls at 0.5 cycles/column.
The x tile holds [fp8(x); fp8(x - fp8(x))] on partition halves, so every
matmul plane contracts w8^T (x_hi + x_lo) -- x at ~2^-8 precision for free.
Each DoubleRow matmul has 2 planes (free column offsets), covering 2 taps.
4 taps use single-fp8 weights (w quantization error ~2.5e-2 * sqrt(4/9));
the other 5 taps split w into fp8 hi+lo planes (error ~1e-3). Host-measured
rel err 1.63e-2 vs the 2e-2 gate. 7 matmuls per 512-column block replaces
the fp16 baseline's 6 at double the per-column rate (3.5 vs 6 cyc/col).
PSUM drain (fp32 -> fp16) alternates between DVE and Act engines.
"""

import numpy as np

import concourse.bacc as bacc
import concourse.tile as tile
from concourse import mybir, bass
from concourse.bass_utils import run_bass_kernel_spmd

# Problem constants (hardcoded per harness contract).
B, C, H, W = 32, 64, 112, 112
OC, KH, KW = 128, 3, 3
NCORES = 8
BPC = B // NCORES          # images per core
HP, WP = H + 2, W + 2      # host-padded input height/width (114)
XFLAT = HP * WP            # 12996 flat padded-input elements per channel
OFLAT = H * WP             # 12768 flat padded-output elements per channel
BLK = 512                  # matmul free-dim block (= 1 PSUM bank of fp32)
NBLK = (OFLAT + BLK - 1) // BLK  # 25 blocks (24 full + 1 of 480)
XBUF = XFLAT + 8           # SBUF image stride (tap reads run to XFLAT+2)
GS = 8                     # out-DMA grouping: 8 blocks per transfer

# matmul dtype for legacy fp16 variants.
MM_DTYPE = mybir.dt.float32r

_cache = {}

# Variant switch:
#   "dr<k>" (k in 0,2,4) - fp8 e4m3 DoubleRow. k taps with single-fp8
#        weights, 9-k taps with hi+lo split weights. Host-measured rel err:
#        dr0 ~1e-3 / dr2 ~1.2e-2 / dr4 ~1.6e-2 (gate 2e-2).
#        Matmuls per block: (18-k)/2.
#   "pack6k128_fp16" - previous fp16 baseline (6 K=128 MMs per block).
#   "pack6", "pack6_bf16", "pack6k128_bf16", "wsplit9_bf16", "wsplit9_ldw",
#   "mm_only", "dma_only" - legacy/probe variants (see git history).
VARIANT = "dr4"

# fp16 output for fp16/fp8 variants (host upcasts); halves out-DMA traffic.
OUT_FP16 = True


def _dr_k(variant):
    return int(variant[2:]) if variant.startswith("dr") else None


# DoubleRow plane-pair plans. Each MM is ((tap0, var0), (tap1, var1));
# var 0 = fp8(w), var 1 = fp8(w - fp8(w)). Taps not appearing with var 1
# use single-fp8 weights. HW CONSTRAINT: the within-partition stride
# between the two planes (s1 = off(tap1) - off(tap0)) must be EVEN --
# odd strides hard-fault the PE. Tap offsets dh*114+dw have parity dw%2,
# so every pair stays within one column-parity class (s1 in {2,114,228}).
_DR_PLANS = {
    # k=4: singles {0,2,1,4}, splits {3,5,6,8} (even) + {7} (odd).
    4: [
        ((0, 0), (2, 0)),
        ((1, 0), (7, 0)),
        ((4, 0), (7, 1)),
        ((3, 0), (5, 0)),
        ((3, 1), (5, 1)),
        ((6, 0), (8, 0)),
        ((6, 1), (8, 1)),
    ],
    # k=2: singles {1,4}.
    2: [
        ((1, 0), (7, 0)),
        ((4, 0), (7, 1)),
        ((0, 0), (2, 0)),
        ((0, 1), (2, 1)),
        ((3, 0), (5, 0)),
        ((3, 1), (5, 1)),
        ((6, 0), (8, 0)),
        ((6, 1), (8, 1)),
    ],
    # k=0: all taps split.
    0: [
        ((1, 0), (4, 0)),
        ((1, 1), (7, 0)),
        ((4, 1), (7, 1)),
        ((0, 0), (2, 0)),
        ((0, 1), (2, 1)),
        ((3, 0), (5, 0)),
        ((3, 1), (5, 1)),
        ((6, 0), (8, 0)),
        ((6, 1), (8, 1)),
    ],
}


def _dr_plan(k):
    plan = _DR_PLANS[k]
    for a, b in plan:
        s1 = _tap_off(b[0]) - _tap_off(a[0])
        assert s1 > 0 and s1 % 2 == 0, (a, b, s1)
    return plan


def _tap_off(t):
    dh, dw = divmod(t, KW)
    return dh * WP + dw


def _build(repeat=1):
    """Build + compile the per-core Bass program (cached per process).

    repeat>1 runs the whole per-core conv `repeat` times back-to-back inside
    one NEFF (idempotent) -- used by test.py to measure steady-state device
    time net of dispatch overhead.
    """
    key = ("nc", repeat, VARIANT)
    if key in _cache:
        return _cache[key]
    variant = VARIANT
    k = _dr_k(variant)
    if k is not None:
        nc = _build_dr(k, repeat)
        _cache[key] = nc
        return nc

    nc = bacc.Bacc("TRN2", target_bir_lowering=False, debug=False)
    if variant.endswith("fp16"):
        mm_dt = mybir.dt.float16
    elif variant.endswith("bf16") or variant in ("mm_only", "wsplit9_ldw"):
        mm_dt = mybir.dt.bfloat16
    else:
        mm_dt = MM_DTYPE
    nslot = 9 if variant in ("wsplit9_bf16", "mm_only", "wsplit9_ldw") else 6
    assert variant in (
        "pack6",
        "pack6_bf16",
        "pack6k128_bf16",
        "pack6k128_fp16",
        "wsplit9_bf16",
        "wsplit9_ldw",
        "mm_only",
        "dma_only",
    ), variant
    # x arrives pre-doubled from the host: per image a [128, XBUF] block whose
    # partitions 0-63 hold the padded image (rows 0-113) and partitions 64-127
    # the same image shifted one row (pack6*) or repeated (wsplit9), so one
    # full-width DMA loads both copies.
    x_ap = nc.dram_tensor(
        "x", [BPC, 2 * C, XBUF], mm_dt, kind="ExternalInput"
    ).ap()
    w_ap = nc.dram_tensor(
        "w", [2 * C, nslot * OC], mm_dt, kind="ExternalInput"
    ).ap()
    out_dt = (
        mybir.dt.float16
        if (OUT_FP16 and mm_dt == mybir.dt.float16)
        else mybir.dt.float32
    )
    out_ap = nc.dram_tensor(
        "out", [BPC, OC, H, WP], out_dt, kind="ExternalOutput"
    ).ap()

    with tile.TileContext(nc) as tc:
        with (
            tc.tile_pool(name="xpool", bufs=3) as xpool,
            tc.tile_pool(name="wpool", bufs=1) as wpool,
            tc.tile_pool(name="opool", bufs=4) as opool,
            tc.tile_pool(name="psum", bufs=8, space="PSUM") as pspool,
        ):
            wt = wpool.tile([2 * C, nslot * OC], mm_dt)
            nc.sync.dma_start(wt[:], w_ap[:])

            def conv_pass():
                for im in range(BPC):
                    xt = xpool.tile([2 * C, XBUF], mm_dt)
                    nc.sync.dma_start(xt[:], x_ap[im])
                    o_im = out_ap[im].rearrange("o h w -> o (h w)")

                    ot = None
                    for blk in range(NBLK):
                        j0 = blk * BLK
                        n = min(BLK, OFLAT - j0)
                        g = blk % GS
                        if g == 0:
                            ot = opool.tile([OC, GS * BLK], out_dt)
                            g0 = j0
                        if variant == "dma_only":
                            nc.vector.tensor_copy(
                                ot[:, g * BLK : g * BLK + n], xt[:OC, j0 : j0 + n]
                            )
                        elif variant in ("wsplit9_bf16", "mm_only"):
                            ps = pspool.tile([OC, BLK], mybir.dt.float32)
                            for t in range(KH * KW):
                                dh, dw = divmod(t, KW)
                                off = j0 + dh * WP + dw
                                nc.tensor.matmul(
                                    ps[:, :n],
                                    lhsT=wt[:, t * OC : (t + 1) * OC],
                                    rhs=xt[:, off : off + n],
                                    start=(t == 0),
                                    stop=(t == KH * KW - 1),
                                )
                            if variant == "mm_only":
                                continue
                            nc.vector.tensor_copy(
                                ot[:, g * BLK : g * BLK + n], ps[:, :n]
                            )
                        else:
                            ps = pspool.tile([OC, BLK], mybir.dt.float32)
                            k128 = variant.startswith("pack6k128")
                            for d in range(3):
                                nc.tensor.matmul(
                                    ps[:, :n],
                                    lhsT=wt[:, d * OC : (d + 1) * OC],
                                    rhs=xt[:, j0 + d : j0 + d + n],
                                    start=(d == 0),
                                    stop=False,
                                )
                            for d in range(3):
                                if k128:
                                    nc.tensor.matmul(
                                        ps[:, :n],
                                        lhsT=wt[:, (3 + d) * OC : (4 + d) * OC],
                                        rhs=xt[:, j0 + WP + d : j0 + WP + d + n],
                                        start=False,
                                        stop=(d == 2),
                                    )
                                else:
                                    nc.tensor.matmul(
                                        ps[:, :n],
                                        lhsT=wt[:C, (3 + d) * OC : (4 + d) * OC],
                                        rhs=xt[
                                            :C,
                                            j0 + 2 * WP + d : j0 + 2 * WP + d + n,
                                        ],
                                        start=False,
                                        stop=(d == 2),
                                    )
                            nc.vector.tensor_copy(
                                ot[:, g * BLK : g * BLK + n], ps[:, :n]
                            )
                        if g == GS - 1 or blk == NBLK - 1:
                            gn = j0 + n - g0
                            nc.sync.dma_start(
                                o_im[:, g0 : g0 + gn], ot[:, :gn]
                            )

            if repeat == 1:
                conv_pass()
            else:
                with tc.For_i(0, repeat, 1):
                    conv_pass()

    nc.compile()
    _cache[key] = nc
    return nc


def _build_dr(k, repeat=1):
    """fp8 e4m3 DoubleRow variant: see module docstring."""
    fp8 = mybir.dt.float8e4
    plan = _dr_plan(k)
    nmm = len(plan)
    out_dt = mybir.dt.float16 if OUT_FP16 else mybir.dt.float32

    nc = bacc.Bacc("TRN2", target_bir_lowering=False, debug=False)
    # Partitions 0-63: fp8(x) padded flat; 64-127: fp8 residual of the same.
    x_ap = nc.dram_tensor("x", [BPC, 2 * C, XBUF], fp8, kind="ExternalInput").ap()
    w_ap = nc.dram_tensor(
        "w", [2 * C, nmm * 2 * OC], fp8, kind="ExternalInput"
    ).ap()
    out_ap = nc.dram_tensor(
        "out", [BPC, OC, H, WP], out_dt, kind="ExternalOutput"
    ).ap()

    with tile.TileContext(nc) as tc:
        with (
            tc.tile_pool(name="xpool", bufs=3) as xpool,
            tc.tile_pool(name="wpool", bufs=1) as wpool,
            tc.tile_pool(name="opool", bufs=4) as opool,
            tc.tile_pool(name="psum", bufs=8, space="PSUM") as pspool,
        ):
            wt = wpool.tile([2 * C, nmm, 2, OC], fp8)
            nc.sync.dma_start(
                wt[:], w_ap[:].rearrange("p (m two o) -> p m two o", m=nmm, two=2)
            )

            def conv_pass():
                for im in range(BPC):
                    xt = xpool.tile([2 * C, XBUF], fp8)
                    nc.sync.dma_start(xt[:], x_ap[im])
                    xfull = xt[:]
                    xtensor, xoff, xpstride = (
                        xfull.tensor,
                        xfull.offset,
                        xfull.ap[0][0],
                    )
                    o_im = out_ap[im].rearrange("o h w -> o (h w)")

                    ot = None
                    for blk in range(NBLK):
                        j0 = blk * BLK
                        n = min(BLK, OFLAT - j0)
                        g = blk % GS
                        if g == 0:
                            ot = opool.tile([OC, GS * BLK], out_dt)
                            g0 = j0
                        ps = pspool.tile([OC, BLK], mybir.dt.float32)
                        for m, (a, b) in enumerate(plan):
                            o0 = _tap_off(a[0])
                            s1 = _tap_off(b[0]) - o0
                            rhs = bass.AP(
                                xtensor,
                                xoff + j0 + o0,
                                [[xpstride, 2 * C], [s1, 2], [1, n]],
                            )
                            nc.tensor.matmul(
                                ps[:, :n],
                                lhsT=wt[:, m],
                                rhs=rhs,
                                start=(m == 0),
                                stop=(m == nmm - 1),
                                perf_mode=mybir.MatmulPerfMode.DoubleRow,
                            )
                        # PSUM -> SBUF fp16 drain, alternating DVE / Act so
                        # neither engine becomes the bottleneck.
                        dst = ot[:, g * BLK : g * BLK + n]
                        if blk % 2 == 0:
                            nc.vector.tensor_copy(dst, ps[:, :n])
                        else:
                            nc.scalar.copy(dst, ps[:, :n])
                        if g == GS - 1 or blk == NBLK - 1:
                            gn = j0 + n - g0
                            nc.sync.dma_start(o_im[:, g0 : g0 + gn], ot[:, :gn])

            if repeat == 1:
                conv_pass()
            else:
                with tc.For_i(0, repeat, 1):
                    conv_pass()

    nc.compile()
    return nc


def run_on_device(nc, in_maps):
    """Single-exec jitted runner with device-resident inputs; returns a
    callable for repeated timing plus the output fetcher."""
    from jax.sharding import Mesh, NamedSharding, PartitionSpec
    from jax.experimental.shard_map import shard_map
    import jax

    from concourse.bass2jax import (
        _bass_exec_p,
        install_neuronx_cc_hook,
        partition_id_tensor,
    )

    install_neuronx_cc_hook()

    partition_name = nc.partition_id_tensor.name if nc.partition_id_tensor else None
    in_names, out_names, out_avals = [], [], []
    for alloc in nc.m.functions[0].allocations:
        if not isinstance(alloc, mybir.MemoryLocationSet):
            continue
        name = alloc.memorylocations[0].name
        if alloc.kind == "ExternalInput":
            if name != partition_name:
                in_names.append(name)
        elif alloc.kind == "ExternalOutput":
            out_names.append(name)
            out_avals.append(
                jax.core.ShapedArray(
                    tuple(alloc.tensor_shape), mybir.dt.np(alloc.dtype)
                )
            )
    n_params = len(in_names)
    all_in_names = list(in_names) + list(out_names)
    if partition_name is not None:
        all_in_names.append(partition_name)
    all_in_names = tuple(all_in_names)

    def body(*args):
        operands = list(args)
        if partition_name is not None:
            operands.append(partition_id_tensor())
        return tuple(
            _bass_exec_p.bind(
                *operands,
                out_avals=tuple(out_avals),
                in_names=all_in_names,
                out_names=tuple(out_names),
                lowering_input_output_aliases=(),
                sim_require_finite=True,
                sim_require_nnan=True,
                nc=nc,
            )
        )

    n_cores = len(in_maps)
    devices = jax.devices()[:n_cores]
    mesh = Mesh(np.asarray(devices), ("core",))
    nspecs = n_params + len(out_names)
    sharded = jax.jit(
        shard_map(
            body,
            mesh=mesh,
            in_specs=(PartitionSpec("core"),) * nspecs,
            out_specs=(PartitionSpec("core"),) * len(out_names),
            check_rep=False,
        )
    )
    concat_in = [
        np.concatenate([np.asarray(in_maps[c][nm]) for c in range(n_cores)], axis=0)
        for nm in in_names
    ]
    concat_zeros = [
        np.zeros((n_cores * a.shape[0], *a.shape[1:]), a.dtype) for a in out_avals
    ]
    sharding = NamedSharding(mesh, PartitionSpec("core"))
    dev_in = [jax.device_put(a, sharding) for a in concat_in]
    dev_zeros = [jax.device_put(a, sharding) for a in concat_zeros]

    def run():
        return sharded(*dev_in, *dev_zeros)

    return run, out_names, out_avals


def _prep_inputs_dr(x, weight, k):
    import ml_dtypes

    FP8 = ml_dtypes.float8_e4m3
    plan = _dr_plan(k)
    nmm = len(plan)

    xp = np.zeros((B, C, HP, WP), dtype=np.float32)
    xp[:, :, 1 : H + 1, 1 : W + 1] = x
    flat = xp.reshape(B, C, XFLAT)
    hi = flat.astype(FP8)
    lo = (flat - hi.astype(np.float32)).astype(FP8)
    xprep = np.zeros((B, 2 * C, XBUF), dtype=FP8)
    xprep[:, :C, :XFLAT] = hi
    xprep[:, C:, :XFLAT] = lo

    w4 = weight[0]  # (out_c, in_c, kh, kw)
    w_hi = w4.astype(FP8)
    w_lo = (w4.astype(np.float32) - w_hi.astype(np.float32)).astype(FP8)
    wvar = (w_hi, w_lo)
    wp = np.zeros((2 * C, nmm, 2, OC), dtype=FP8)
    for m, pair in enumerate(plan):
        for i, (t, v) in enumerate(pair):
            kh, kw = divmod(t, KW)
            wmat = wvar[v][:, :, kh, kw].T  # (in_c, out_c)
            wp[:C, m, i] = wmat
            wp[C:, m, i] = wmat
    w_prep = np.ascontiguousarray(wp.reshape(2 * C, nmm * 2 * OC))
    return [
        {"x": xprep[c * BPC : (c + 1) * BPC], "w": w_prep} for c in range(NCORES)
    ]


def _prep_inputs(x, weight):
    """Host-side shard + layout prep. Returns per-core input maps."""
    import ml_dtypes

    variant = VARIANT
    k = _dr_k(variant)
    if k is not None:
        return _prep_inputs_dr(x, weight, k)

    if variant.endswith("fp16"):
        host_dt = np.float16
    elif variant.endswith("bf16") or variant in ("mm_only", "wsplit9_ldw"):
        host_dt = ml_dtypes.bfloat16
    else:
        host_dt = np.float32

    xp = np.zeros((B, C, HP, WP), dtype=np.float32)
    xp[:, :, 1 : H + 1, 1 : W + 1] = x
    flat = xp.reshape(B, C, XFLAT).astype(host_dt)
    xprep = np.zeros((B, 2 * C, XBUF), dtype=host_dt)
    xprep[:, :C, :XFLAT] = flat
    if variant in ("wsplit9_bf16", "mm_only", "wsplit9_ldw"):
        xprep[:, C:, :XFLAT] = flat
    else:
        xprep[:, C:, : XFLAT - WP] = flat[:, :, WP:]

    w4 = weight[0]  # (out_c, in_c, kh, kw)
    if variant in ("wsplit9_bf16", "mm_only", "wsplit9_ldw"):
        w_hi = w4.astype(ml_dtypes.bfloat16)
        w_lo = (w4.astype(np.float32) - w_hi.astype(np.float32)).astype(
            ml_dtypes.bfloat16
        )
        wp = np.zeros((2 * C, KH * KW, OC), dtype=host_dt)
        for t in range(KH * KW):
            kh, kw = divmod(t, KW)
            wp[:C, t] = w_hi[:, :, kh, kw].T
            wp[C:, t] = w_lo[:, :, kh, kw].T
        w_prep = np.ascontiguousarray(wp.reshape(2 * C, KH * KW * OC))
    else:
        wp = np.zeros((2 * C, 6, OC), dtype=host_dt)
        for d in range(KW):
            wp[:C, d] = w4[:, :, 0, d].T.astype(host_dt)
            wp[C:, d] = w4[:, :, 1, d].T.astype(host_dt)
            if variant.startswith("pack6k128"):
                wp[C:, 3 + d] = w4[:, :, 2, d].T.astype(host_dt)
            else:
                wp[:C, 3 + d] = w4[:, :, 2, d].T.astype(host_dt)
        w_prep = np.ascontiguousarray(wp.reshape(2 * C, 6 * OC))
    return [
        {"x": xprep[c * BPC : (c + 1) * BPC], "w": w_prep} for c in range(NCORES)
    ]


def kernel(x, weight):
    x = np.asarray(x, dtype=np.float32)
    weight = np.asarray(weight, dtype=np.float32)
    nc = _build()
    in_maps = _prep_inputs(x, weight)
    # Retry on transient device failures (a crashed prior process can leave
    # the first subsequent execution returning UNAVAILABLE or garbage) and
    # validate the output is finite before returning.
    last_exc = None
    for _attempt in range(3):
        try:
            res = run_bass_kernel_spmd(nc, in_maps, list(range(NCORES)))
            out = np.concatenate(
                [res.results[c]["out"] for c in range(NCORES)], axis=0
            )
        except Exception as exc:  # noqa: BLE001 - retry any runtime failure
            last_exc = exc
            continue
        out = np.ascontiguousarray(out[:, :, :, :W].astype(np.float32))
        if np.isfinite(out).all():
            return out
    if last_exc is not None:
        raise last_exc
    return out


# revision 18
# speedup vs baseline: 1.1212x; 1.1212x over previous
"""Trainium2 Bass kernel: 3x3 conv2d (stride 1, pad 1), NCHW.

x (32, 64, 112, 112) f32, weight (1, 128, 64, 3, 3) f32 -> out (32, 128, 112, 112) f32.

Strategy: data-parallel over batch across 8 cores (4 images/core).
Per core, conv is computed as PSUM-accumulating matmuls over kernel taps:
x is host-padded to (114, 114) so each tap's shifted input window is a
constant free-dim offset into the flat [114*114] SBUF image. Output is
produced in padded row-major (112 x 114) layout and sliced on the host.

Default variant "pack5gs4": fp16 operands, 5 all-K=128 matmuls per
512-column block: 3 vertical tap-pair MMs (taps (0,d)+(1,d) via the
one-row-shifted copy on partitions 64-127 of the x tile) plus 2 MMs on
the appended H-columns ([x+2WP; x+2WP+1] on the partition halves)
covering row-2 taps ((2,0)+(2,1) paired, then (2,2) with zero upper
weights). PSUM drains on DVE; out-DMA in groups of 4 blocks (GS=4
measured ~13us faster than GS=8). fp16 output, host upcasts.
Measured ~117us/conv vs the 150us 6-MM GS=8 baseline; rel err ~3.6e-4.

Measured dead ends (this hardware): fp8 DoubleRow runs at ~1 cycle/column
(not the 0.5 the cost model promises; SwInterleave ~same), so fp8 pairing
loses to fp16; 2-block interleaved PE chains are slower than straight
accumulation chains (PSUM bank switching costs ~20ns/MM); tap-major
weight reuse (ldweights=False) saves nothing. DoubleRow ifmap plane
strides must be EVEN or the PE hard-faults.
"""

import numpy as np

import concourse.bacc as bacc
import concourse.tile as tile
from concourse import mybir, bass
from concourse.bass_utils import run_bass_kernel_spmd

# Problem constants (hardcoded per harness contract).
B, C, H, W = 32, 64, 112, 112
OC, KH, KW = 128, 3, 3
NCORES = 8
BPC = B // NCORES          # images per core
HP, WP = H + 2, W + 2      # host-padded input height/width (114)
XFLAT = HP * WP            # 12996 flat padded-input elements per channel
OFLAT = H * WP             # 12768 flat padded-output elements per channel
BLK = 512                  # matmul free-dim block (= 1 PSUM bank of fp32)
NBLK = (OFLAT + BLK - 1) // BLK  # 25 blocks (24 full + 1 of 480)
XBUF = XFLAT + 8           # SBUF image stride (tap reads run to XFLAT+2)
GS = 8                     # out-DMA grouping: 8 blocks per transfer

# matmul dtype for legacy fp16 variants.
MM_DTYPE = mybir.dt.float32r

_cache = {}

# Variant switch:
#   "dr<k>" (k in 0,2,4) - fp8 e4m3 DoubleRow. k taps with single-fp8
#        weights, 9-k taps with hi+lo split weights. Host-measured rel err:
#        dr0 ~1e-3 / dr2 ~1.2e-2 / dr4 ~1.6e-2 (gate 2e-2).
#        Matmuls per block: (18-k)/2.
#   "pack5_fp16" - fp16, 5 K=128 MMs per block: 3 vertical tap-pairs from
#        the [x; x+WP] tile plus 2 MMs from a second [x+2WP; x+2WP+1] tile
#        covering row-2 taps ((2,0)+(2,1) paired, (2,2) with zero upper).
#        Drains alternate DVE/Act.
#   "pack6k128_fp16" - previous fp16 baseline (6 K=128 MMs per block).
#   "pack6", "pack6_bf16", "pack6k128_bf16", "wsplit9_bf16", "wsplit9_ldw",
#   "mm_only", "dma_only" - legacy/probe variants (see git history).
VARIANT = "pack5gs4"

# fp16 output for fp16/fp8 variants (host upcasts); halves out-DMA traffic.
OUT_FP16 = True


def _dr_k(variant):
    return int(variant[2:]) if variant.startswith("dr") else None


# DoubleRow plane-pair plans. Each MM is ((tap0, var0), (tap1, var1));
# var 0 = fp8(w), var 1 = fp8(w - fp8(w)). Taps not appearing with var 1
# use single-fp8 weights. HW CONSTRAINT: the within-partition stride
# between the two planes (s1 = off(tap1) - off(tap0)) must be EVEN --
# odd strides hard-fault the PE. Tap offsets dh*114+dw have parity dw%2,
# so every pair stays within one column-parity class (s1 in {2,114,228}).
_DR_PLANS = {
    # k=4: singles {0,2,1,4}, splits {3,5,6,8} (even) + {7} (odd).
    4: [
        ((0, 0), (2, 0)),
        ((1, 0), (7, 0)),
        ((4, 0), (7, 1)),
        ((3, 0), (5, 0)),
        ((3, 1), (5, 1)),
        ((6, 0), (8, 0)),
        ((6, 1), (8, 1)),
    ],
    # k=2: singles {1,4}.
    2: [
        ((1, 0), (7, 0)),
        ((4, 0), (7, 1)),
        ((0, 0), (2, 0)),
        ((0, 1), (2, 1)),
        ((3, 0), (5, 0)),
        ((3, 1), (5, 1)),
        ((6, 0), (8, 0)),
        ((6, 1), (8, 1)),
    ],
    # k=0: all taps split.
    0: [
        ((1, 0), (4, 0)),
        ((1, 1), (7, 0)),
        ((4, 1), (7, 1)),
        ((0, 0), (2, 0)),
        ((0, 1), (2, 1)),
        ((3, 0), (5, 0)),
        ((3, 1), (5, 1)),
        ((6, 0), (8, 0)),
        ((6, 1), (8, 1)),
    ],
}


def _dr_plan(k):
    plan = _DR_PLANS[k]
    for a, b in plan:
        s1 = _tap_off(b[0]) - _tap_off(a[0])
        assert s1 > 0 and s1 % 2 == 0, (a, b, s1)
    return plan


def _tap_off(t):
    dh, dw = divmod(t, KW)
    return dh * WP + dw


def _build(repeat=1):
    """Build + compile the per-core Bass program (cached per process).

    repeat>1 runs the whole per-core conv `repeat` times back-to-back inside
    one NEFF (idempotent) -- used by test.py to measure steady-state device
    time net of dispatch overhead.
    """
    key = ("nc", repeat, VARIANT)
    if key in _cache:
        return _cache[key]
    variant = VARIANT
    k = _dr_k(variant)
    if k is not None:
        nc = _build_dr(k, repeat)
        _cache[key] = nc
        return nc
    if variant.startswith("pack5"):
        nc = _build_pack5(repeat, gsv=4 if variant == "pack5gs4" else GS)
        _cache[key] = nc
        return nc

    nc = bacc.Bacc("TRN2", target_bir_lowering=False, debug=False)
    if variant.endswith("fp16") or variant.startswith("p6"):
        mm_dt = mybir.dt.float16
    elif variant.endswith("bf16") or variant in ("mm_only", "wsplit9_ldw"):
        mm_dt = mybir.dt.bfloat16
    else:
        mm_dt = MM_DTYPE
    nslot = 9 if variant in ("wsplit9_bf16", "mm_only", "wsplit9_ldw") else 6
    assert variant in (
        "pack6",
        "pack6_bf16",
        "pack6k128_bf16",
        "pack6k128_fp16",
        "pack6k128i_fp16",
        "p6mm",
        "p6mm1x",
        "p6md",
        "p6md1",
        "p6gs4",
        "p6gs2",
        "p6gs4b8",
        "p6gs16",
        "p6tm",
        "p6tm2",
        "wsplit9_bf16",
        "wsplit9_ldw",
        "mm_only",
        "dma_only",
    ), variant
    # x arrives pre-doubled from the host: per image a [128, XBUF] block whose
    # partitions 0-63 hold the padded image (rows 0-113) and partitions 64-127
    # the same image shifted one row (pack6*) or repeated (wsplit9), so one
    # full-width DMA loads both copies.
    x_ap = nc.dram_tensor(
        "x", [BPC, 2 * C, XBUF], mm_dt, kind="ExternalInput"
    ).ap()
    w_ap = nc.dram_tensor(
        "w", [2 * C, nslot * OC], mm_dt, kind="ExternalInput"
    ).ap()
    out_dt = (
        mybir.dt.float16
        if (OUT_FP16 and mm_dt == mybir.dt.float16)
        else mybir.dt.float32
    )
    out_ap = nc.dram_tensor(
        "out", [BPC, OC, H, WP], out_dt, kind="ExternalOutput"
    ).ap()
    assert not (variant == "p6mm" and repeat == 1) or True

    with tile.TileContext(nc) as tc:
        with (
            tc.tile_pool(name="xpool", bufs=3) as xpool,
            tc.tile_pool(name="wpool", bufs=1) as wpool,
            tc.tile_pool(name="opool", bufs=4) as opool,
            tc.tile_pool(name="psum", bufs=8, space="PSUM") as pspool,
        ):
            wt = wpool.tile([2 * C, nslot * OC], mm_dt)
            nc.sync.dma_start(wt[:], w_ap[:])

            def conv_pass_interleaved():
                # Two blocks' accumulation chains interleaved on the PE so
                # consecutive instructions hit different PSUM banks -- back-
                # to-back dependent matmuls otherwise stall ~100ns each on
                # the PE's SBUF-access pipeline refill.
                for im in range(BPC):
                    xt = xpool.tile([2 * C, XBUF], mm_dt)
                    nc.sync.dma_start(xt[:], x_ap[im])
                    o_im = out_ap[im].rearrange("o h w -> o (h w)")

                    ot = None
                    for pb in range(0, NBLK, 2):
                        blks = [b for b in (pb, pb + 1) if b < NBLK]
                        if pb % GS == 0:
                            ot = opool.tile([OC, GS * BLK], out_dt)
                            g0 = pb * BLK
                        pss = [
                            pspool.tile(
                                [OC, BLK], mybir.dt.float32, name="ps", tag="ps"
                            )
                            for _ in blks
                        ]
                        ns = [min(BLK, OFLAT - b * BLK) for b in blks]
                        for m in range(6):
                            for bi, blk in enumerate(blks):
                                j0 = blk * BLK
                                n = ns[bi]
                                if m < 3:
                                    off = j0 + m
                                    w_sl = wt[:, m * OC : (m + 1) * OC]
                                else:
                                    off = j0 + WP + (m - 3)
                                    w_sl = wt[:, m * OC : (m + 1) * OC]
                                nc.tensor.matmul(
                                    pss[bi][:, :n],
                                    lhsT=w_sl,
                                    rhs=xt[:, off : off + n],
                                    start=(m == 0),
                                    stop=(m == 5),
                                )
                        for bi, blk in enumerate(blks):
                            g = blk % GS
                            dst = ot[:, g * BLK : g * BLK + ns[bi]]
                            if bi == 0:
                                nc.vector.tensor_copy(dst, pss[bi][:, : ns[bi]])
                            else:
                                nc.scalar.copy(dst, pss[bi][:, : ns[bi]])
                        lastblk = blks[-1]
                        if lastblk % GS == GS - 1 or lastblk == NBLK - 1:
                            gn = lastblk * BLK + ns[-1] - g0
                            nc.sync.dma_start(o_im[:, g0 : g0 + gn], ot[:, :gn])

            GSV = {
                "p6gs4": 4,
                "p6gs2": 2,
                "p6gs4b8": 4,
                "p6gs16": 16,
                "p6tm": 4,
                "p6tm2": 2,
            }.get(variant, GS)

            def conv_pass_tapmajor(tg):
                # Tap-major over groups of `tg` blocks: one weight load per
                # tap per group, the other tg-1 matmuls reuse the loaded
                # weights (ldweights=False; PE executes in FIFO order).
                # Out-DMA group == tap group. Drains on DVE only.
                for im in range(BPC):
                    xt = xpool.tile([2 * C, XBUF], mm_dt)
                    nc.sync.dma_start(xt[:], x_ap[im])
                    o_im = out_ap[im].rearrange("o h w -> o (h w)")
                    for g0blk in range(0, NBLK, tg):
                        blks = list(range(g0blk, min(g0blk + tg, NBLK)))
                        g0 = g0blk * BLK
                        ot = opool.tile([OC, tg * BLK], out_dt)
                        pss = [
                            pspool.tile(
                                [OC, BLK], mybir.dt.float32, name="ps", tag="ps"
                            )
                            for _ in blks
                        ]
                        ns = [min(BLK, OFLAT - b * BLK) for b in blks]
                        for m in range(6):
                            for bi, blk in enumerate(blks):
                                j0 = blk * BLK
                                off = j0 + m if m < 3 else j0 + WP + (m - 3)
                                mm = nc.tensor.matmul(
                                    pss[bi][:, : ns[bi]],
                                    lhsT=wt[:, m * OC : (m + 1) * OC],
                                    rhs=xt[:, off : off + ns[bi]],
                                    start=(m == 0),
                                    stop=(m == 5),
                                )
                                if bi > 0:
                                    mm.ldweights = False
                        for bi, blk in enumerate(blks):
                            dst = ot[:, bi * BLK : bi * BLK + ns[bi]]
                            nc.vector.tensor_copy(dst, pss[bi][:, : ns[bi]])
                        gn = blks[-1] * BLK + ns[-1] - g0
                        nc.sync.dma_start(o_im[:, g0 : g0 + gn], ot[:, :gn])

            def conv_pass():
                if variant == "pack6k128i_fp16":
                    conv_pass_interleaved()
                    return
                if variant in ("p6tm", "p6tm2"):
                    conv_pass_tapmajor(GSV)
                    return
                xt_shared = None
                for im in range(BPC):
                    if variant == "p6mm1x":
                        if xt_shared is None:
                            xt_shared = xpool.tile([2 * C, XBUF], mm_dt)
                            nc.sync.dma_start(xt_shared[:], x_ap[0])
                        xt = xt_shared
                    else:
                        xt = xpool.tile([2 * C, XBUF], mm_dt)
                        nc.sync.dma_start(xt[:], x_ap[im])
                    o_im = out_ap[im].rearrange("o h w -> o (h w)")

                    ot = None
                    for blk in range(NBLK):
                        j0 = blk * BLK
                        n = min(BLK, OFLAT - j0)
                        g = blk % GSV
                        if g == 0:
                            ot = opool.tile(
                                [OC, GSV * BLK],
                                out_dt,
                                bufs=8 if variant == "p6gs4b8" else None,
                            )
                            g0 = j0
                        if variant == "dma_only":
                            nc.vector.tensor_copy(
                                ot[:, g * BLK : g * BLK + n], xt[:OC, j0 : j0 + n]
                            )
                        elif variant in ("wsplit9_bf16", "mm_only"):
                            ps = pspool.tile([OC, BLK], mybir.dt.float32)
                            for t in range(KH * KW):
                                dh, dw = divmod(t, KW)
                                off = j0 + dh * WP + dw
                                nc.tensor.matmul(
                                    ps[:, :n],
                                    lhsT=wt[:, t * OC : (t + 1) * OC],
                                    rhs=xt[:, off : off + n],
                                    start=(t == 0),
                                    stop=(t == KH * KW - 1),
                                )
                            if variant == "mm_only":
                                continue
                            nc.vector.tensor_copy(
                                ot[:, g * BLK : g * BLK + n], ps[:, :n]
                            )
                        else:
                            ps = pspool.tile([OC, BLK], mybir.dt.float32)
                            k128 = variant.startswith(("pack6k128", "p6"))
                            for d in range(3):
                                nc.tensor.matmul(
                                    ps[:, :n],
                                    lhsT=wt[:, d * OC : (d + 1) * OC],
                                    rhs=xt[:, j0 + d : j0 + d + n],
                                    start=(d == 0),
                                    stop=False,
                                )
                            for d in range(3):
                                if k128:
                                    nc.tensor.matmul(
                                        ps[:, :n],
                                        lhsT=wt[:, (3 + d) * OC : (4 + d) * OC],
                                        rhs=xt[:, j0 + WP + d : j0 + WP + d + n],
                                        start=False,
                                        stop=(d == 2),
                                    )
                                else:
                                    nc.tensor.matmul(
                                        ps[:, :n],
                                        lhsT=wt[:C, (3 + d) * OC : (4 + d) * OC],
                                        rhs=xt[
                                            :C,
                                            j0 + 2 * WP + d : j0 + 2 * WP + d + n,
                                        ],
                                        start=False,
                                        stop=(d == 2),
                                    )
                            if variant in ("p6mm", "p6mm1x"):
                                continue
                            dst = ot[:, g * BLK : g * BLK + n]
                            if variant == "p6md" and blk % 2:
                                nc.scalar.copy(dst, ps[:, :n])
                            else:
                                nc.vector.tensor_copy(dst, ps[:, :n])
                            if variant in ("p6md", "p6md1"):
                                continue
                        if g == GSV - 1 or blk == NBLK - 1:
                            gn = j0 + n - g0
                            nc.sync.dma_start(
                                o_im[:, g0 : g0 + gn], ot[:, :gn]
                            )

            if repeat == 1:
                conv_pass()
            else:
                with tc.For_i(0, repeat, 1):
                    conv_pass()

    nc.compile()
    _cache[key] = nc
    return nc


OBUF = OFLAT + 4  # H-tile (row-2) columns per partition for pack5


def _build_pack5(repeat=1, gsv=GS):
    """fp16 5-MM variant: 3 vertical tap-pair MMs from [x; x+WP] plus 2 MMs
    from the appended [x+2WP; x+2WP+1] columns covering row-2 taps."""
    fp16 = mybir.dt.float16
    out_dt = mybir.dt.float16 if OUT_FP16 else mybir.dt.float32
    XW = XBUF + OBUF

    nc = bacc.Bacc("TRN2", target_bir_lowering=False, debug=False)
    x_ap = nc.dram_tensor("x", [BPC, 2 * C, XW], fp16, kind="ExternalInput").ap()
    w_ap = nc.dram_tensor("w", [2 * C, 5 * OC], fp16, kind="ExternalInput").ap()
    out_ap = nc.dram_tensor(
        "out", [BPC, OC, H, WP], out_dt, kind="ExternalOutput"
    ).ap()

    with tile.TileContext(nc) as tc:
        with (
            tc.tile_pool(name="xpool", bufs=2) as xpool,
            tc.tile_pool(name="wpool", bufs=1) as wpool,
            tc.tile_pool(name="opool", bufs=4) as opool,
            tc.tile_pool(name="psum", bufs=8, space="PSUM") as pspool,
        ):
            wt = wpool.tile([2 * C, 5 * OC], fp16)
            nc.sync.dma_start(wt[:], w_ap[:])

            def conv_pass():
                for im in range(BPC):
                    xt = xpool.tile([2 * C, XW], fp16)
                    nc.sync.dma_start(xt[:], x_ap[im])
                    o_im = out_ap[im].rearrange("o h w -> o (h w)")

                    ot = None
                    for blk in range(NBLK):
                        j0 = blk * BLK
                        n = min(BLK, OFLAT - j0)
                        g = blk % gsv
                        if g == 0:
                            ot = opool.tile([OC, gsv * BLK], out_dt)
                            g0 = j0
                        ps = pspool.tile([OC, BLK], mybir.dt.float32)
                        for d in range(3):
                            nc.tensor.matmul(
                                ps[:, :n],
                                lhsT=wt[:, d * OC : (d + 1) * OC],
                                rhs=xt[:, j0 + d : j0 + d + n],
                                start=(d == 0),
                                stop=False,
                            )
                        nc.tensor.matmul(
                            ps[:, :n],
                            lhsT=wt[:, 3 * OC : 4 * OC],
                            rhs=xt[:, XBUF + j0 : XBUF + j0 + n],
                            start=False,
                            stop=False,
                        )
                        nc.tensor.matmul(
                            ps[:, :n],
                            lhsT=wt[:, 4 * OC : 5 * OC],
                            rhs=xt[:, XBUF + j0 + 2 : XBUF + j0 + 2 + n],
                            start=False,
                            stop=True,
                        )
                        dst = ot[:, g * BLK : g * BLK + n]
                        nc.vector.tensor_copy(dst, ps[:, :n])
                        if g == gsv - 1 or blk == NBLK - 1:
                            gn = j0 + n - g0
                            nc.sync.dma_start(o_im[:, g0 : g0 + gn], ot[:, :gn])

            if repeat == 1:
                conv_pass()
            else:
                with tc.For_i(0, repeat, 1):
                    conv_pass()

    nc.compile()
    return nc


def _prep_inputs_pack5(x, weight):
    xp = np.zeros((B, C, HP, WP), dtype=np.float32)
    xp[:, :, 1 : H + 1, 1 : W + 1] = x
    flat = xp.reshape(B, C, XFLAT).astype(np.float16)
    XW = XBUF + OBUF
    xprep = np.zeros((B, 2 * C, XW), dtype=np.float16)
    xprep[:, :C, :XFLAT] = flat
    xprep[:, C:, : XFLAT - WP] = flat[:, :, WP:]
    nh = XFLAT - 2 * WP  # valid columns from x+2WP
    xprep[:, :C, XBUF : XBUF + nh] = flat[:, :, 2 * WP :]
    xprep[:, C:, XBUF : XBUF + nh - 1] = flat[:, :, 2 * WP + 1 :]

    w4 = weight[0].astype(np.float16)  # (out_c, in_c, kh, kw)
    wp = np.zeros((2 * C, 5, OC), dtype=np.float16)
    for d in range(KW):
        wp[:C, d] = w4[:, :, 0, d].T
        wp[C:, d] = w4[:, :, 1, d].T
    wp[:C, 3] = w4[:, :, 2, 0].T
    wp[C:, 3] = w4[:, :, 2, 1].T
    wp[:C, 4] = w4[:, :, 2, 2].T
    w_prep = np.ascontiguousarray(wp.reshape(2 * C, 5 * OC))
    return [
        {"x": xprep[c * BPC : (c + 1) * BPC], "w": w_prep} for c in range(NCORES)
    ]


def _build_dr(k, repeat=1):
    """fp8 e4m3 DoubleRow variant: see module docstring."""
    fp8 = mybir.dt.float8e4
    plan = _dr_plan(k)
    nmm = len(plan)
    out_dt = mybir.dt.float16 if OUT_FP16 else mybir.dt.float32

    nc = bacc.Bacc("TRN2", target_bir_lowering=False, debug=False)
    # Partitions 0-63: fp8(x) padded flat; 64-127: fp8 residual of the same.
    x_ap = nc.dram_tensor("x", [BPC, 2 * C, XBUF], fp8, kind="ExternalInput").ap()
    w_ap = nc.dram_tensor(
        "w", [2 * C, nmm * 2 * OC], fp8, kind="ExternalInput"
    ).ap()
    out_ap = nc.dram_tensor(
        "out", [BPC, OC, H, WP], out_dt, kind="ExternalOutput"
    ).ap()

    with tile.TileContext(nc) as tc:
        with (
            tc.tile_pool(name="xpool", bufs=3) as xpool,
            tc.tile_pool(name="wpool", bufs=1) as wpool,
            tc.tile_pool(name="opool", bufs=4) as opool,
            tc.tile_pool(name="psum", bufs=8, space="PSUM") as pspool,
        ):
            wt = wpool.tile([2 * C, nmm, 2, OC], fp8)
            nc.sync.dma_start(
                wt[:], w_ap[:].rearrange("p (m two o) -> p m two o", m=nmm, two=2)
            )

            def conv_pass():
                for im in range(BPC):
                    xt = xpool.tile([2 * C, XBUF], fp8)
                    nc.sync.dma_start(xt[:], x_ap[im])
                    xfull = xt[:]
                    xtensor, xoff, xpstride = (
                        xfull.tensor,
                        xfull.offset,
                        xfull.ap[0][0],
                    )
                    o_im = out_ap[im].rearrange("o h w -> o (h w)")

                    ot = None
                    for blk in range(NBLK):
                        j0 = blk * BLK
                        n = min(BLK, OFLAT - j0)
                        g = blk % GS
                        if g == 0:
                            ot = opool.tile([OC, GS * BLK], out_dt)
                            g0 = j0
                        ps = pspool.tile([OC, BLK], mybir.dt.float32)
                        for m, (a, b) in enumerate(plan):
                            o0 = _tap_off(a[0])
                            s1 = _tap_off(b[0]) - o0
                            rhs = bass.AP(
                                xtensor,
                                xoff + j0 + o0,
                                [[xpstride, 2 * C], [s1, 2], [1, n]],
                            )
                            nc.tensor.matmul(
                                ps[:, :n],
                                lhsT=wt[:, m],
                                rhs=rhs,
                                start=(m == 0),
                                stop=(m == nmm - 1),
                                perf_mode=mybir.MatmulPerfMode.DoubleRow,
                            )
                        # PSUM -> SBUF fp16 drain, alternating DVE / Act so
                        # neither engine becomes the bottleneck.
                        dst = ot[:, g * BLK : g * BLK + n]
                        if blk % 2 == 0:
                            nc.vector.tensor_copy(dst, ps[:, :n])
                        else:
                            nc.scalar.copy(dst, ps[:, :n])
                        if g == GS - 1 or blk == NBLK - 1:
                            gn = j0 + n - g0
                            nc.sync.dma_start(o_im[:, g0 : g0 + gn], ot[:, :gn])

            if repeat == 1:
                conv_pass()
            else:
                with tc.For_i(0, repeat, 1):
                    conv_pass()

    nc.compile()
    return nc


def run_on_device(nc, in_maps):
    """Single-exec jitted runner with device-resident inputs; returns a
    callable for repeated timing plus the output fetcher."""
    from jax.sharding import Mesh, NamedSharding, PartitionSpec
    from jax.experimental.shard_map import shard_map
    import jax

    from concourse.bass2jax import (
        _bass_exec_p,
        install_neuronx_cc_hook,
        partition_id_tensor,
    )

    install_neuronx_cc_hook()

    partition_name = nc.partition_id_tensor.name if nc.partition_id_tensor else None
    in_names, out_names, out_avals = [], [], []
    for alloc in nc.m.functions[0].allocations:
        if not isinstance(alloc, mybir.MemoryLocationSet):
            continue
        name = alloc.memorylocations[0].name
        if alloc.kind == "ExternalInput":
            if name != partition_name:
                in_names.append(name)
        elif alloc.kind == "ExternalOutput":
            out_names.append(name)
            out_avals.append(
                jax.core.ShapedArray(
                    tuple(alloc.tensor_shape), mybir.dt.np(alloc.dtype)
                )
            )
    n_params = len(in_names)
    all_in_names = list(in_names) + list(out_names)
    if partition_name is not None:
        all_in_names.append(partition_name)
    all_in_names = tuple(all_in_names)

    def body(*args):
        operands = list(args)
        if partition_name is not None:
            operands.append(partition_id_tensor())
        return tuple(
            _bass_exec_p.bind(
                *operands,
                out_avals=tuple(out_avals),
                in_names=all_in_names,
                out_names=tuple(out_names),
                lowering_input_output_aliases=(),
                sim_require_finite=True,
                sim_require_nnan=True,
                nc=nc,
            )
        )

    n_cores = len(in_maps)
    devices = jax.devices()[:n_cores]
    mesh = Mesh(np.asarray(devices), ("core",))
    nspecs = n_params + len(out_names)
    sharded = jax.jit(
        shard_map(
            body,
            mesh=mesh,
            in_specs=(PartitionSpec("core"),) * nspecs,
            out_specs=(PartitionSpec("core"),) * len(out_names),
            check_rep=False,
        )
    )
    concat_in = [
        np.concatenate([np.asarray(in_maps[c][nm]) for c in range(n_cores)], axis=0)
        for nm in in_names
    ]
    concat_zeros = [
        np.zeros((n_cores * a.shape[0], *a.shape[1:]), a.dtype) for a in out_avals
    ]
    sharding = NamedSharding(mesh, PartitionSpec("core"))
    dev_in = [jax.device_put(a, sharding) for a in concat_in]
    dev_zeros = [jax.device_put(a, sharding) for a in concat_zeros]

    def run():
        return sharded(*dev_in, *dev_zeros)

    return run, out_names, out_avals


def _prep_inputs_dr(x, weight, k):
    import ml_dtypes

    FP8 = ml_dtypes.float8_e4m3
    plan = _dr_plan(k)
    nmm = len(plan)

    xp = np.zeros((B, C, HP, WP), dtype=np.float32)
    xp[:, :, 1 : H + 1, 1 : W + 1] = x
    flat = xp.reshape(B, C, XFLAT)
    hi = flat.astype(FP8)
    lo = (flat - hi.astype(np.float32)).astype(FP8)
    xprep = np.zeros((B, 2 * C, XBUF), dtype=FP8)
    xprep[:, :C, :XFLAT] = hi
    xprep[:, C:, :XFLAT] = lo

    w4 = weight[0]  # (out_c, in_c, kh, kw)
    w_hi = w4.astype(FP8)
    w_lo = (w4.astype(np.float32) - w_hi.astype(np.float32)).astype(FP8)
    wvar = (w_hi, w_lo)
    wp = np.zeros((2 * C, nmm, 2, OC), dtype=FP8)
    for m, pair in enumerate(plan):
        for i, (t, v) in enumerate(pair):
            kh, kw = divmod(t, KW)
            wmat = wvar[v][:, :, kh, kw].T  # (in_c, out_c)
            wp[:C, m, i] = wmat
            wp[C:, m, i] = wmat
    w_prep = np.ascontiguousarray(wp.reshape(2 * C, nmm * 2 * OC))
    return [
        {"x": xprep[c * BPC : (c + 1) * BPC], "w": w_prep} for c in range(NCORES)
    ]


def _prep_inputs(x, weight):
    """Host-side shard + layout prep. Returns per-core input maps."""
    import ml_dtypes

    variant = VARIANT
    k = _dr_k(variant)
    if k is not None:
        return _prep_inputs_dr(x, weight, k)
    if variant.startswith("pack5"):
        return _prep_inputs_pack5(x, weight)

    if variant.endswith("fp16"):
        host_dt = np.float16
    elif variant.endswith("bf16") or variant in ("mm_only", "wsplit9_ldw"):
        host_dt = ml_dtypes.bfloat16
    else:
        host_dt = np.float32

    xp = np.zeros((B, C, HP, WP), dtype=np.float32)
    xp[:, :, 1 : H + 1, 1 : W + 1] = x
    flat = xp.reshape(B, C, XFLAT).astype(host_dt)
    xprep = np.zeros((B, 2 * C, XBUF), dtype=host_dt)
    xprep[:, :C, :XFLAT] = flat
    if variant in ("wsplit9_bf16", "mm_only", "wsplit9_ldw"):
        xprep[:, C:, :XFLAT] = flat
    else:
        xprep[:, C:, : XFLAT - WP] = flat[:, :, WP:]

    w4 = weight[0]  # (out_c, in_c, kh, kw)
    if variant in ("wsplit9_bf16", "mm_only", "wsplit9_ldw"):
        w_hi = w4.astype(ml_dtypes.bfloat16)
        w_lo = (w4.astype(np.float32) - w_hi.astype(np.float32)).astype(
            ml_dtypes.bfloat16
        )
        wp = np.zeros((2 * C, KH * KW, OC), dtype=host_dt)
        for t in range(KH * KW):
            kh, kw = divmod(t, KW)
            wp[:C, t] = w_hi[:, :, kh, kw].T
            wp[C:, t] = w_lo[:, :, kh, kw].T
        w_prep = np.ascontiguousarray(wp.reshape(2 * C, KH * KW * OC))
    else:
        wp = np.zeros((2 * C, 6, OC), dtype=host_dt)
        for d in range(KW):
            wp[:C, d] = w4[:, :, 0, d].T.astype(host_dt)
            wp[C:, d] = w4[:, :, 1, d].T.astype(host_dt)
            if variant.startswith("pack6k128"):
                wp[C:, 3 + d] = w4[:, :, 2, d].T.astype(host_dt)
            else:
                wp[:C, 3 + d] = w4[:, :, 2, d].T.astype(host_dt)
        w_prep = np.ascontiguousarray(wp.reshape(2 * C, 6 * OC))
    return [
        {"x": xprep[c * BPC : (c + 1) * BPC], "w": w_prep} for c in range(NCORES)
    ]


def kernel(x, weight):
    x = np.asarray(x, dtype=np.float32)
    weight = np.asarray(weight, dtype=np.float32)
    nc = _build()
    in_maps = _prep_inputs(x, weight)
    # Retry on transient device failures (a crashed prior process can leave
    # the first subsequent execution returning UNAVAILABLE or garbage) and
    # validate the output is finite before returning.
    last_exc = None
    for _attempt in range(3):
        try:
            res = run_bass_kernel_spmd(nc, in_maps, list(range(NCORES)))
            out = np.concatenate(
                [res.results[c]["out"] for c in range(NCORES)], axis=0
            )
        except Exception as exc:  # noqa: BLE001 - retry any runtime failure
            last_exc = exc
            continue
        out = np.ascontiguousarray(out[:, :, :, :W].astype(np.float32))
        if np.isfinite(out).all():
            return out
    if last_exc is not None:
        raise last_exc
    return out


# revision 20
# speedup vs baseline: 1.3374x; 1.1928x over previous
"""Trainium2 Bass kernel: 3x3 conv2d (stride 1, pad 1), NCHW.

x (32, 64, 112, 112) f32, weight (1, 128, 64, 3, 3) f32 -> out (32, 128, 112, 112) f32.

Strategy: data-parallel over batch across 8 cores (4 images/core).
Per core, conv is computed as PSUM-accumulating matmuls over kernel taps:
x is host-padded to (114, 114) so each tap's shifted input window is a
constant free-dim offset into the flat [114*114] SBUF image. Output is
produced in padded row-major (112 x 114) layout and sliced on the host.

Default variant "pack5gs4": fp16 operands, 5 all-K=128 matmuls per
512-column block: 3 vertical tap-pair MMs (taps (0,d)+(1,d) via the
one-row-shifted copy on partitions 64-127 of the x tile) plus 2 MMs on
the appended H-columns ([x+2WP; x+2WP+1] on the partition halves)
covering row-2 taps ((2,0)+(2,1) paired, then (2,2) with zero upper
weights). PSUM drains on DVE; out-DMA in groups of 4 blocks (GS=4
measured ~13us faster than GS=8). fp16 output, host upcasts.
Measured ~117us/conv vs the 150us 6-MM GS=8 baseline; rel err ~3.6e-4.

Measured dead ends (this hardware): fp8 DoubleRow runs at ~1 cycle/column
(not the 0.5 the cost model promises; SwInterleave ~same), so fp8 pairing
loses to fp16; 2-block interleaved PE chains are slower than straight
accumulation chains (PSUM bank switching costs ~20ns/MM); tap-major
weight reuse (ldweights=False) saves nothing. DoubleRow ifmap plane
strides must be EVEN or the PE hard-faults.
"""

import numpy as np

import concourse.bacc as bacc
import concourse.tile as tile
from concourse import mybir, bass
from concourse.bass_utils import run_bass_kernel_spmd

# Problem constants (hardcoded per harness contract).
B, C, H, W = 32, 64, 112, 112
OC, KH, KW = 128, 3, 3
NCORES = 8
BPC = B // NCORES          # images per core
HP, WP = H + 2, W + 2      # host-padded input height/width (114)
XFLAT = HP * WP            # 12996 flat padded-input elements per channel
OFLAT = H * WP             # 12768 flat padded-output elements per channel
BLK = 512                  # matmul free-dim block (= 1 PSUM bank of fp32)
NBLK = (OFLAT + BLK - 1) // BLK  # 25 blocks (24 full + 1 of 480)
XBUF = XFLAT + 8           # SBUF image stride (tap reads run to XFLAT+2)
GS = 8                     # out-DMA grouping: 8 blocks per transfer

# matmul dtype for legacy fp16 variants.
MM_DTYPE = mybir.dt.float32r

_cache = {}

# Variant switch:
#   "dr<k>" (k in 0,2,4) - fp8 e4m3 DoubleRow. k taps with single-fp8
#        weights, 9-k taps with hi+lo split weights. Host-measured rel err:
#        dr0 ~1e-3 / dr2 ~1.2e-2 / dr4 ~1.6e-2 (gate 2e-2).
#        Matmuls per block: (18-k)/2.
#   "pack5_fp16" - fp16, 5 K=128 MMs per block: 3 vertical tap-pairs from
#        the [x; x+WP] tile plus 2 MMs from a second [x+2WP; x+2WP+1] tile
#        covering row-2 taps ((2,0)+(2,1) paired, (2,2) with zero upper).
#        Drains alternate DVE/Act.
#   "pack6k128_fp16" - previous fp16 baseline (6 K=128 MMs per block).
#   "pack6", "pack6_bf16", "pack6k128_bf16", "wsplit9_bf16", "wsplit9_ldw",
#   "mm_only", "dma_only" - legacy/probe variants (see git history).
VARIANT = "pack5gs4"

# fp16 output for fp16/fp8 variants (host upcasts); halves out-DMA traffic.
OUT_FP16 = True


def _dr_k(variant):
    return int(variant[2:]) if variant.startswith("dr") else None


# DoubleRow plane-pair plans. Each MM is ((tap0, var0), (tap1, var1));
# var 0 = fp8(w), var 1 = fp8(w - fp8(w)). Taps not appearing with var 1
# use single-fp8 weights. HW CONSTRAINT: the within-partition stride
# between the two planes (s1 = off(tap1) - off(tap0)) must be EVEN --
# odd strides hard-fault the PE. Tap offsets dh*114+dw have parity dw%2,
# so every pair stays within one column-parity class (s1 in {2,114,228}).
_DR_PLANS = {
    # k=4: singles {0,2,1,4}, splits {3,5,6,8} (even) + {7} (odd).
    4: [
        ((0, 0), (2, 0)),
        ((1, 0), (7, 0)),
        ((4, 0), (7, 1)),
        ((3, 0), (5, 0)),
        ((3, 1), (5, 1)),
        ((6, 0), (8, 0)),
        ((6, 1), (8, 1)),
    ],
    # k=2: singles {1,4}.
    2: [
        ((1, 0), (7, 0)),
        ((4, 0), (7, 1)),
        ((0, 0), (2, 0)),
        ((0, 1), (2, 1)),
        ((3, 0), (5, 0)),
        ((3, 1), (5, 1)),
        ((6, 0), (8, 0)),
        ((6, 1), (8, 1)),
    ],
    # k=0: all taps split.
    0: [
        ((1, 0), (4, 0)),
        ((1, 1), (7, 0)),
        ((4, 1), (7, 1)),
        ((0, 0), (2, 0)),
        ((0, 1), (2, 1)),
        ((3, 0), (5, 0)),
        ((3, 1), (5, 1)),
        ((6, 0), (8, 0)),
        ((6, 1), (8, 1)),
    ],
}


def _dr_plan(k):
    plan = _DR_PLANS[k]
    for a, b in plan:
        s1 = _tap_off(b[0]) - _tap_off(a[0])
        assert s1 > 0 and s1 % 2 == 0, (a, b, s1)
    return plan


def _tap_off(t):
    dh, dw = divmod(t, KW)
    return dh * WP + dw


def _build(repeat=1):
    """Build + compile the per-core Bass program (cached per process).

    repeat>1 runs the whole per-core conv `repeat` times back-to-back inside
    one NEFF (idempotent) -- used by test.py to measure steady-state device
    time net of dispatch overhead.
    """
    key = ("nc", repeat, VARIANT)
    if key in _cache:
        return _cache[key]
    variant = VARIANT
    k = _dr_k(variant)
    if k is not None:
        nc = _build_dr(k, repeat)
        _cache[key] = nc
        return nc
    if variant.startswith("pack5") or variant == "p5mm":
        mode = ""
        gsv = GS
        if variant in ("pack5gs4", "pack5s", "pack5b3", "pack5sb3", "p5mm"):
            gsv = 4
        if variant == "pack5gs5":
            gsv = 5
        if variant == "p5mm":
            mode = "mm"
        elif variant == "pack5s":
            mode = "s"
        elif variant == "pack5b3":
            mode = "b3"
        elif variant == "pack5sb3":
            mode = "sb3"
        nc = _build_pack5(repeat, gsv=gsv, mode=mode)
        _cache[key] = nc
        return nc

    nc = bacc.Bacc("TRN2", target_bir_lowering=False, debug=False)
    if variant.endswith("fp16") or variant.startswith("p6"):
        mm_dt = mybir.dt.float16
    elif variant.endswith("bf16") or variant in ("mm_only", "wsplit9_ldw"):
        mm_dt = mybir.dt.bfloat16
    else:
        mm_dt = MM_DTYPE
    nslot = 9 if variant in ("wsplit9_bf16", "mm_only", "wsplit9_ldw") else 6
    assert variant in (
        "pack6",
        "pack6_bf16",
        "pack6k128_bf16",
        "pack6k128_fp16",
        "pack6k128i_fp16",
        "p6mm",
        "p6mm1x",
        "p6md",
        "p6md1",
        "p6gs4",
        "p6gs2",
        "p6gs4b8",
        "p6gs16",
        "p6tm",
        "p6tm2",
        "wsplit9_bf16",
        "wsplit9_ldw",
        "mm_only",
        "dma_only",
    ), variant
    # x arrives pre-doubled from the host: per image a [128, XBUF] block whose
    # partitions 0-63 hold the padded image (rows 0-113) and partitions 64-127
    # the same image shifted one row (pack6*) or repeated (wsplit9), so one
    # full-width DMA loads both copies.
    x_ap = nc.dram_tensor(
        "x", [BPC, 2 * C, XBUF], mm_dt, kind="ExternalInput"
    ).ap()
    w_ap = nc.dram_tensor(
        "w", [2 * C, nslot * OC], mm_dt, kind="ExternalInput"
    ).ap()
    out_dt = (
        mybir.dt.float16
        if (OUT_FP16 and mm_dt == mybir.dt.float16)
        else mybir.dt.float32
    )
    out_ap = nc.dram_tensor(
        "out", [BPC, OC, H, WP], out_dt, kind="ExternalOutput"
    ).ap()
    assert not (variant == "p6mm" and repeat == 1) or True

    with tile.TileContext(nc) as tc:
        with (
            tc.tile_pool(name="xpool", bufs=3) as xpool,
            tc.tile_pool(name="wpool", bufs=1) as wpool,
            tc.tile_pool(name="opool", bufs=4) as opool,
            tc.tile_pool(name="psum", bufs=8, space="PSUM") as pspool,
        ):
            wt = wpool.tile([2 * C, nslot * OC], mm_dt)
            nc.sync.dma_start(wt[:], w_ap[:])

            def conv_pass_interleaved():
                # Two blocks' accumulation chains interleaved on the PE so
                # consecutive instructions hit different PSUM banks -- back-
                # to-back dependent matmuls otherwise stall ~100ns each on
                # the PE's SBUF-access pipeline refill.
                for im in range(BPC):
                    xt = xpool.tile([2 * C, XBUF], mm_dt)
                    nc.sync.dma_start(xt[:], x_ap[im])
                    o_im = out_ap[im].rearrange("o h w -> o (h w)")

                    ot = None
                    for pb in range(0, NBLK, 2):
                        blks = [b for b in (pb, pb + 1) if b < NBLK]
                        if pb % GS == 0:
                            ot = opool.tile([OC, GS * BLK], out_dt)
                            g0 = pb * BLK
                        pss = [
                            pspool.tile(
                                [OC, BLK], mybir.dt.float32, name="ps", tag="ps"
                            )
                            for _ in blks
                        ]
                        ns = [min(BLK, OFLAT - b * BLK) for b in blks]
                        for m in range(6):
                            for bi, blk in enumerate(blks):
                                j0 = blk * BLK
                                n = ns[bi]
                                if m < 3:
                                    off = j0 + m
                                    w_sl = wt[:, m * OC : (m + 1) * OC]
                                else:
                                    off = j0 + WP + (m - 3)
                                    w_sl = wt[:, m * OC : (m + 1) * OC]
                                nc.tensor.matmul(
                                    pss[bi][:, :n],
                                    lhsT=w_sl,
                                    rhs=xt[:, off : off + n],
                                    start=(m == 0),
                                    stop=(m == 5),
                                )
                        for bi, blk in enumerate(blks):
                            g = blk % GS
                            dst = ot[:, g * BLK : g * BLK + ns[bi]]
                            if bi == 0:
                                nc.vector.tensor_copy(dst, pss[bi][:, : ns[bi]])
                            else:
                                nc.scalar.copy(dst, pss[bi][:, : ns[bi]])
                        lastblk = blks[-1]
                        if lastblk % GS == GS - 1 or lastblk == NBLK - 1:
                            gn = lastblk * BLK + ns[-1] - g0
                            nc.sync.dma_start(o_im[:, g0 : g0 + gn], ot[:, :gn])

            GSV = {
                "p6gs4": 4,
                "p6gs2": 2,
                "p6gs4b8": 4,
                "p6gs16": 16,
                "p6tm": 4,
                "p6tm2": 2,
            }.get(variant, GS)

            def conv_pass_tapmajor(tg):
                # Tap-major over groups of `tg` blocks: one weight load per
                # tap per group, the other tg-1 matmuls reuse the loaded
                # weights (ldweights=False; PE executes in FIFO order).
                # Out-DMA group == tap group. Drains on DVE only.
                for im in range(BPC):
                    xt = xpool.tile([2 * C, XBUF], mm_dt)
                    nc.sync.dma_start(xt[:], x_ap[im])
                    o_im = out_ap[im].rearrange("o h w -> o (h w)")
                    for g0blk in range(0, NBLK, tg):
                        blks = list(range(g0blk, min(g0blk + tg, NBLK)))
                        g0 = g0blk * BLK
                        ot = opool.tile([OC, tg * BLK], out_dt)
                        pss = [
                            pspool.tile(
                                [OC, BLK], mybir.dt.float32, name="ps", tag="ps"
                            )
                            for _ in blks
                        ]
                        ns = [min(BLK, OFLAT - b * BLK) for b in blks]
                        for m in range(6):
                            for bi, blk in enumerate(blks):
                                j0 = blk * BLK
                                off = j0 + m if m < 3 else j0 + WP + (m - 3)
                                mm = nc.tensor.matmul(
                                    pss[bi][:, : ns[bi]],
                                    lhsT=wt[:, m * OC : (m + 1) * OC],
                                    rhs=xt[:, off : off + ns[bi]],
                                    start=(m == 0),
                                    stop=(m == 5),
                                )
                                if bi > 0:
                                    mm.ldweights = False
                        for bi, blk in enumerate(blks):
                            dst = ot[:, bi * BLK : bi * BLK + ns[bi]]
                            nc.vector.tensor_copy(dst, pss[bi][:, : ns[bi]])
                        gn = blks[-1] * BLK + ns[-1] - g0
                        nc.sync.dma_start(o_im[:, g0 : g0 + gn], ot[:, :gn])

            def conv_pass():
                if variant == "pack6k128i_fp16":
                    conv_pass_interleaved()
                    return
                if variant in ("p6tm", "p6tm2"):
                    conv_pass_tapmajor(GSV)
                    return
                xt_shared = None
                for im in range(BPC):
                    if variant == "p6mm1x":
                        if xt_shared is None:
                            xt_shared = xpool.tile([2 * C, XBUF], mm_dt)
                            nc.sync.dma_start(xt_shared[:], x_ap[0])
                        xt = xt_shared
                    else:
                        xt = xpool.tile([2 * C, XBUF], mm_dt)
                        nc.sync.dma_start(xt[:], x_ap[im])
                    o_im = out_ap[im].rearrange("o h w -> o (h w)")

                    ot = None
                    for blk in range(NBLK):
                        j0 = blk * BLK
                        n = min(BLK, OFLAT - j0)
                        g = blk % GSV
                        if g == 0:
                            ot = opool.tile(
                                [OC, GSV * BLK],
                                out_dt,
                                bufs=8 if variant == "p6gs4b8" else None,
                            )
                            g0 = j0
                        if variant == "dma_only":
                            nc.vector.tensor_copy(
                                ot[:, g * BLK : g * BLK + n], xt[:OC, j0 : j0 + n]
                            )
                        elif variant in ("wsplit9_bf16", "mm_only"):
                            ps = pspool.tile([OC, BLK], mybir.dt.float32)
                            for t in range(KH * KW):
                                dh, dw = divmod(t, KW)
                                off = j0 + dh * WP + dw
                                nc.tensor.matmul(
                                    ps[:, :n],
                                    lhsT=wt[:, t * OC : (t + 1) * OC],
                                    rhs=xt[:, off : off + n],
                                    start=(t == 0),
                                    stop=(t == KH * KW - 1),
                                )
                            if variant == "mm_only":
                                continue
                            nc.vector.tensor_copy(
                                ot[:, g * BLK : g * BLK + n], ps[:, :n]
                            )
                        else:
                            ps = pspool.tile([OC, BLK], mybir.dt.float32)
                            k128 = variant.startswith(("pack6k128", "p6"))
                            for d in range(3):
                                nc.tensor.matmul(
                                    ps[:, :n],
                                    lhsT=wt[:, d * OC : (d + 1) * OC],
                                    rhs=xt[:, j0 + d : j0 + d + n],
                                    start=(d == 0),
                                    stop=False,
                                )
                            for d in range(3):
                                if k128:
                                    nc.tensor.matmul(
                                        ps[:, :n],
                                        lhsT=wt[:, (3 + d) * OC : (4 + d) * OC],
                                        rhs=xt[:, j0 + WP + d : j0 + WP + d + n],
                                        start=False,
                                        stop=(d == 2),
                                    )
                                else:
                                    nc.tensor.matmul(
                                        ps[:, :n],
                                        lhsT=wt[:C, (3 + d) * OC : (4 + d) * OC],
                                        rhs=xt[
                                            :C,
                                            j0 + 2 * WP + d : j0 + 2 * WP + d + n,
                                        ],
                                        start=False,
                                        stop=(d == 2),
                                    )
                            if variant in ("p6mm", "p6mm1x"):
                                continue
                            dst = ot[:, g * BLK : g * BLK + n]
                            if variant == "p6md" and blk % 2:
                                nc.scalar.copy(dst, ps[:, :n])
                            else:
                                nc.vector.tensor_copy(dst, ps[:, :n])
                            if variant in ("p6md", "p6md1"):
                                continue
                        if g == GSV - 1 or blk == NBLK - 1:
                            gn = j0 + n - g0
                            nc.sync.dma_start(
                                o_im[:, g0 : g0 + gn], ot[:, :gn]
                            )

            if repeat == 1:
                conv_pass()
            else:
                with tc.For_i(0, repeat, 1):
                    conv_pass()

    nc.compile()
    _cache[key] = nc
    return nc


OBUF = OFLAT + 4  # H-tile (row-2) columns per partition for pack5


def _build_pack5(repeat=1, gsv=GS, mode=""):
    """fp16 5-MM variant: 3 vertical tap-pair MMs from [x; x+WP] plus 2 MMs
    from the appended [x+2WP; x+2WP+1] columns covering row-2 taps."""
    fp16 = mybir.dt.float16
    out_dt = mybir.dt.float16 if OUT_FP16 else mybir.dt.float32
    XW = XBUF + OBUF

    nc = bacc.Bacc("TRN2", target_bir_lowering=False, debug=False)
    xw_dram = XBUF if "s" in mode else XW
    x_ap = nc.dram_tensor(
        "x", [BPC, 2 * C, xw_dram], fp16, kind="ExternalInput"
    ).ap()
    w_ap = nc.dram_tensor("w", [2 * C, 5 * OC], fp16, kind="ExternalInput").ap()
    out_ap = nc.dram_tensor(
        "out", [BPC, OC, H, WP], out_dt, kind="ExternalOutput"
    ).ap()

    with tile.TileContext(nc) as tc:
        with (
            tc.tile_pool(name="xpool", bufs=3 if "b3" in mode else 2) as xpool,
            tc.tile_pool(name="wpool", bufs=1) as wpool,
            tc.tile_pool(name="opool", bufs=4) as opool,
            tc.tile_pool(name="psum", bufs=8, space="PSUM") as pspool,
        ):
            wt = wpool.tile([2 * C, 5 * OC], fp16)
            nc.sync.dma_start(wt[:], w_ap[:])

            def conv_pass():
                nh = OFLAT + 2
                for im in range(BPC):
                    xt = xpool.tile([2 * C, XW], fp16)
                    if "s" in mode:
                        # HBM carries only the [x; x+WP] columns; the row-2
                        # H-columns are built by two same-partition SBUF->SBUF
                        # column-shift copies (halves HBM x traffic).
                        nc.sync.dma_start(xt[:, :XBUF], x_ap[im])
                        nc.sync.dma_start(
                            xt[:C, XBUF : XBUF + nh],
                            xt[:C, 2 * WP : 2 * WP + nh],
                        )
                        nc.sync.dma_start(
                            xt[C:, XBUF : XBUF + nh],
                            xt[C:, WP + 1 : WP + 1 + nh],
                        )
                    else:
                        nc.sync.dma_start(xt[:], x_ap[im])
                    o_im = out_ap[im].rearrange("o h w -> o (h w)")

                    ot = None
                    for blk in range(NBLK):
                        j0 = blk * BLK
                        n = min(BLK, OFLAT - j0)
                        g = blk % gsv
                        if g == 0:
                            ot = opool.tile([OC, gsv * BLK], out_dt)
                            g0 = j0
                        ps = pspool.tile([OC, BLK], mybir.dt.float32)
                        for d in range(3):
                            nc.tensor.matmul(
                                ps[:, :n],
                                lhsT=wt[:, d * OC : (d + 1) * OC],
                                rhs=xt[:, j0 + d : j0 + d + n],
                                start=(d == 0),
                                stop=False,
                            )
                        nc.tensor.matmul(
                            ps[:, :n],
                            lhsT=wt[:, 3 * OC : 4 * OC],
                            rhs=xt[:, XBUF + j0 : XBUF + j0 + n],
                            start=False,
                            stop=False,
                        )
                        nc.tensor.matmul(
                            ps[:, :n],
                            lhsT=wt[:, 4 * OC : 5 * OC],
                            rhs=xt[:, XBUF + j0 + 2 : XBUF + j0 + 2 + n],
                            start=False,
                            stop=True,
                        )
                        if "mm" in mode:
                            continue
                        dst = ot[:, g * BLK : g * BLK + n]
                        nc.vector.tensor_copy(dst, ps[:, :n])
                        if g == gsv - 1 or blk == NBLK - 1:
                            gn = j0 + n - g0
                            nc.sync.dma_start(o_im[:, g0 : g0 + gn], ot[:, :gn])

            if repeat == 1:
                conv_pass()
            else:
                with tc.For_i(0, repeat, 1):
                    conv_pass()

    nc.compile()
    return nc


def _prep_inputs_pack5(x, weight, v_only=False):
    xp = np.zeros((B, C, HP, WP), dtype=np.float32)
    xp[:, :, 1 : H + 1, 1 : W + 1] = x
    flat = xp.reshape(B, C, XFLAT).astype(np.float16)
    XW = XBUF if v_only else XBUF + OBUF
    xprep = np.zeros((B, 2 * C, XW), dtype=np.float16)
    xprep[:, :C, :XFLAT] = flat
    xprep[:, C:, : XFLAT - WP] = flat[:, :, WP:]
    if not v_only:
        nh = XFLAT - 2 * WP  # valid columns from x+2WP
        xprep[:, :C, XBUF : XBUF + nh] = flat[:, :, 2 * WP :]
        xprep[:, C:, XBUF : XBUF + nh - 1] = flat[:, :, 2 * WP + 1 :]

    w4 = weight[0].astype(np.float16)  # (out_c, in_c, kh, kw)
    wp = np.zeros((2 * C, 5, OC), dtype=np.float16)
    for d in range(KW):
        wp[:C, d] = w4[:, :, 0, d].T
        wp[C:, d] = w4[:, :, 1, d].T
    wp[:C, 3] = w4[:, :, 2, 0].T
    wp[C:, 3] = w4[:, :, 2, 1].T
    wp[:C, 4] = w4[:, :, 2, 2].T
    w_prep = np.ascontiguousarray(wp.reshape(2 * C, 5 * OC))
    return [
        {"x": xprep[c * BPC : (c + 1) * BPC], "w": w_prep} for c in range(NCORES)
    ]


def _build_dr(k, repeat=1):
    """fp8 e4m3 DoubleRow variant: see module docstring."""
    fp8 = mybir.dt.float8e4
    plan = _dr_plan(k)
    nmm = len(plan)
    out_dt = mybir.dt.float16 if OUT_FP16 else mybir.dt.float32

    nc = bacc.Bacc("TRN2", target_bir_lowering=False, debug=False)
    # Partitions 0-63: fp8(x) padded flat; 64-127: fp8 residual of the same.
    x_ap = nc.dram_tensor("x", [BPC, 2 * C, XBUF], fp8, kind="ExternalInput").ap()
    w_ap = nc.dram_tensor(
        "w", [2 * C, nmm * 2 * OC], fp8, kind="ExternalInput"
    ).ap()
    out_ap = nc.dram_tensor(
        "out", [BPC, OC, H, WP], out_dt, kind="ExternalOutput"
    ).ap()

    with tile.TileContext(nc) as tc:
        with (
            tc.tile_pool(name="xpool", bufs=3) as xpool,
            tc.tile_pool(name="wpool", bufs=1) as wpool,
            tc.tile_pool(name="opool", bufs=4) as opool,
            tc.tile_pool(name="psum", bufs=8, space="PSUM") as pspool,
        ):
            wt = wpool.tile([2 * C, nmm, 2, OC], fp8)
            nc.sync.dma_start(
                wt[:], w_ap[:].rearrange("p (m two o) -> p m two o", m=nmm, two=2)
            )

            def conv_pass():
                for im in range(BPC):
                    xt = xpool.tile([2 * C, XBUF], fp8)
                    nc.sync.dma_start(xt[:], x_ap[im])
                    xfull = xt[:]
                    xtensor, xoff, xpstride = (
                        xfull.tensor,
                        xfull.offset,
                        xfull.ap[0][0],
                    )
                    o_im = out_ap[im].rearrange("o h w -> o (h w)")

                    ot = None
                    for blk in range(NBLK):
                        j0 = blk * BLK
                        n = min(BLK, OFLAT - j0)
                        g = blk % GS
                        if g == 0:
                            ot = opool.tile([OC, GS * BLK], out_dt)
                            g0 = j0
                        ps = pspool.tile([OC, BLK], mybir.dt.float32)
                        for m, (a, b) in enumerate(plan):
                            o0 = _tap_off(a[0])
                            s1 = _tap_off(b[0]) - o0
                            rhs = bass.AP(
                                xtensor,
                                xoff + j0 + o0,
                                [[xpstride, 2 * C], [s1, 2], [1, n]],
                            )
                            nc.tensor.matmul(
                                ps[:, :n],
                                lhsT=wt[:, m],
                                rhs=rhs,
                                start=(m == 0),
                                stop=(m == nmm - 1),
                                perf_mode=mybir.MatmulPerfMode.DoubleRow,
                            )
                        # PSUM -> SBUF fp16 drain, alternating DVE / Act so
                        # neither engine becomes the bottleneck.
                        dst = ot[:, g * BLK : g * BLK + n]
                        if blk % 2 == 0:
                            nc.vector.tensor_copy(dst, ps[:, :n])
                        else:
                            nc.scalar.copy(dst, ps[:, :n])
                        if g == GS - 1 or blk == NBLK - 1:
                            gn = j0 + n - g0
                            nc.sync.dma_start(o_im[:, g0 : g0 + gn], ot[:, :gn])

            if repeat == 1:
                conv_pass()
            else:
                with tc.For_i(0, repeat, 1):
                    conv_pass()

    nc.compile()
    return nc


def run_on_device(nc, in_maps):
    """Single-exec jitted runner with device-resident inputs; returns a
    callable for repeated timing plus the output fetcher."""
    from jax.sharding import Mesh, NamedSharding, PartitionSpec
    from jax.experimental.shard_map import shard_map
    import jax

    from concourse.bass2jax import (
        _bass_exec_p,
        install_neuronx_cc_hook,
        partition_id_tensor,
    )

    install_neuronx_cc_hook()

    partition_name = nc.partition_id_tensor.name if nc.partition_id_tensor else None
    in_names, out_names, out_avals = [], [], []
    for alloc in nc.m.functions[0].allocations:
        if not isinstance(alloc, mybir.MemoryLocationSet):
            continue
        name = alloc.memorylocations[0].name
        if alloc.kind == "ExternalInput":
            if name != partition_name:
                in_names.append(name)
        elif alloc.kind == "ExternalOutput":
            out_names.append(name)
            out_avals.append(
                jax.core.ShapedArray(
                    tuple(alloc.tensor_shape), mybir.dt.np(alloc.dtype)
                )
            )
    n_params = len(in_names)
    all_in_names = list(in_names) + list(out_names)
    if partition_name is not None:
        all_in_names.append(partition_name)
    all_in_names = tuple(all_in_names)

    def body(*args):
        operands = list(args)
        if partition_name is not None:
            operands.append(partition_id_tensor())
        return tuple(
            _bass_exec_p.bind(
                *operands,
                out_avals=tuple(out_avals),
                in_names=all_in_names,
                out_names=tuple(out_names),
                lowering_input_output_aliases=(),
                sim_require_finite=True,
                sim_require_nnan=True,
                nc=nc,
            )
        )

    n_cores = len(in_maps)
    devices = jax.devices()[:n_cores]
    mesh = Mesh(np.asarray(devices), ("core",))
    nspecs = n_params + len(out_names)
    sharded = jax.jit(
        shard_map(
            body,
            mesh=mesh,
            in_specs=(PartitionSpec("core"),) * nspecs,
            out_specs=(PartitionSpec("core"),) * len(out_names),
            check_rep=False,
        )
    )
    concat_in = [
        np.concatenate([np.asarray(in_maps[c][nm]) for c in range(n_cores)], axis=0)
        for nm in in_names
    ]
    concat_zeros = [
        np.zeros((n_cores * a.shape[0], *a.shape[1:]), a.dtype) for a in out_avals
    ]
    sharding = NamedSharding(mesh, PartitionSpec("core"))
    dev_in = [jax.device_put(a, sharding) for a in concat_in]
    dev_zeros = [jax.device_put(a, sharding) for a in concat_zeros]

    def run():
        return sharded(*dev_in, *dev_zeros)

    return run, out_names, out_avals


def _prep_inputs_dr(x, weight, k):
    import ml_dtypes

    FP8 = ml_dtypes.float8_e4m3
    plan = _dr_plan(k)
    nmm = len(plan)

    xp = np.zeros((B, C, HP, WP), dtype=np.float32)
    xp[:, :, 1 : H + 1, 1 : W + 1] = x
    flat = xp.reshape(B, C, XFLAT)
    hi = flat.astype(FP8)
    lo = (flat - hi.astype(np.float32)).astype(FP8)
    xprep = np.zeros((B, 2 * C, XBUF), dtype=FP8)
    xprep[:, :C, :XFLAT] = hi
    xprep[:, C:, :XFLAT] = lo

    w4 = weight[0]  # (out_c, in_c, kh, kw)
    w_hi = w4.astype(FP8)
    w_lo = (w4.astype(np.float32) - w_hi.astype(np.float32)).astype(FP8)
    wvar = (w_hi, w_lo)
    wp = np.zeros((2 * C, nmm, 2, OC), dtype=FP8)
    for m, pair in enumerate(plan):
        for i, (t, v) in enumerate(pair):
            kh, kw = divmod(t, KW)
            wmat = wvar[v][:, :, kh, kw].T  # (in_c, out_c)
            wp[:C, m, i] = wmat
            wp[C:, m, i] = wmat
    w_prep = np.ascontiguousarray(wp.reshape(2 * C, nmm * 2 * OC))
    return [
        {"x": xprep[c * BPC : (c + 1) * BPC], "w": w_prep} for c in range(NCORES)
    ]


def _prep_inputs(x, weight):
    """Host-side shard + layout prep. Returns per-core input maps."""
    import ml_dtypes

    variant = VARIANT
    k = _dr_k(variant)
    if k is not None:
        return _prep_inputs_dr(x, weight, k)
    if variant.startswith("pack5") or variant == "p5mm":
        return _prep_inputs_pack5(
            x, weight, v_only=(variant in ("pack5s", "pack5sb3"))
        )

    if variant.endswith("fp16"):
        host_dt = np.float16
    elif variant.endswith("bf16") or variant in ("mm_only", "wsplit9_ldw"):
        host_dt = ml_dtypes.bfloat16
    else:
        host_dt = np.float32

    xp = np.zeros((B, C, HP, WP), dtype=np.float32)
    xp[:, :, 1 : H + 1, 1 : W + 1] = x
    flat = xp.reshape(B, C, XFLAT).astype(host_dt)
    xprep = np.zeros((B, 2 * C, XBUF), dtype=host_dt)
    xprep[:, :C, :XFLAT] = flat
    if variant in ("wsplit9_bf16", "mm_only", "wsplit9_ldw"):
        xprep[:, C:, :XFLAT] = flat
    else:
        xprep[:, C:, : XFLAT - WP] = flat[:, :, WP:]

    w4 = weight[0]  # (out_c, in_c, kh, kw)
    if variant in ("wsplit9_bf16", "mm_only", "wsplit9_ldw"):
        w_hi = w4.astype(ml_dtypes.bfloat16)
        w_lo = (w4.astype(np.float32) - w_hi.astype(np.float32)).astype(
            ml_dtypes.bfloat16
        )
        wp = np.zeros((2 * C, KH * KW, OC), dtype=host_dt)
        for t in range(KH * KW):
            kh, kw = divmod(t, KW)
            wp[:C, t] = w_hi[:, :, kh, kw].T
            wp[C:, t] = w_lo[:, :, kh, kw].T
        w_prep = np.ascontiguousarray(wp.reshape(2 * C, KH * KW * OC))
    else:
        wp = np.zeros((2 * C, 6, OC), dtype=host_dt)
        for d in range(KW):
            wp[:C, d] = w4[:, :, 0, d].T.astype(host_dt)
            wp[C:, d] = w4[:, :, 1, d].T.astype(host_dt)
            if variant.startswith("pack6k128"):
                wp[C:, 3 + d] = w4[:, :, 2, d].T.astype(host_dt)
            else:
                wp[:C, 3 + d] = w4[:, :, 2, d].T.astype(host_dt)
        w_prep = np.ascontiguousarray(wp.reshape(2 * C, 6 * OC))
    return [
        {"x": xprep[c * BPC : (c + 1) * BPC], "w": w_prep} for c in range(NCORES)
    ]


def kernel(x, weight):
    x = np.asarray(x, dtype=np.float32)
    weight = np.asarray(weight, dtype=np.float32)
    nc = _build()
    in_maps = _prep_inputs(x, weight)
    # Retry on transient device failures (a crashed prior process can leave
    # the first subsequent execution returning UNAVAILABLE or garbage) and
    # validate the output is finite before returning.
    last_exc = None
    for _attempt in range(3):
        try:
            res = run_bass_kernel_spmd(nc, in_maps, list(range(NCORES)))
            out = np.concatenate(
                [res.results[c]["out"] for c in range(NCORES)], axis=0
            )
        except Exception as exc:  # noqa: BLE001 - retry any runtime failure
            last_exc = exc
            continue
        out = np.ascontiguousarray(out[:, :, :, :W].astype(np.float32))
        if np.isfinite(out).all():
            return out
    if last_exc is not None:
        raise last_exc
    return out


# revision 21
# speedup vs baseline: 1.3619x; 1.0183x over previous
"""Trainium2 Bass kernel: 3x3 conv2d (stride 1, pad 1), NCHW.

x (32, 64, 112, 112) f32, weight (1, 128, 64, 3, 3) f32 -> out (32, 128, 112, 112) f32.

Strategy: data-parallel over batch across 8 cores (4 images/core).
Per core, conv is computed as PSUM-accumulating matmuls over kernel taps:
x is host-padded to (114, 114) so each tap's shifted input window is a
constant free-dim offset into the flat [114*114] SBUF image. Output is
produced in padded row-major (112 x 114) layout and sliced on the host.

Default variant "pack5gs4": fp16 operands, 5 all-K=128 matmuls per
512-column block: 3 vertical tap-pair MMs (taps (0,d)+(1,d) via the
one-row-shifted copy on partitions 64-127 of the x tile) plus 2 MMs on
the appended H-columns ([x+2WP; x+2WP+1] on the partition halves)
covering row-2 taps ((2,0)+(2,1) paired, then (2,2) with zero upper
weights). PSUM drains on DVE; out-DMA in groups of 4 blocks (GS=4
measured ~13us faster than GS=8). fp16 output, host upcasts.
Measured ~117us/conv vs the 150us 6-MM GS=8 baseline; rel err ~3.6e-4.

Measured dead ends (this hardware): fp8 DoubleRow runs at ~1 cycle/column
(not the 0.5 the cost model promises; SwInterleave ~same), so fp8 pairing
loses to fp16; 2-block interleaved PE chains are slower than straight
accumulation chains (PSUM bank switching costs ~20ns/MM); tap-major
weight reuse (ldweights=False) saves nothing. DoubleRow ifmap plane
strides must be EVEN or the PE hard-faults.
"""

import numpy as np

import concourse.bacc as bacc
import concourse.tile as tile
from concourse import mybir, bass
from concourse.bass_utils import run_bass_kernel_spmd

# Problem constants (hardcoded per harness contract).
B, C, H, W = 32, 64, 112, 112
OC, KH, KW = 128, 3, 3
NCORES = 8
BPC = B // NCORES          # images per core
HP, WP = H + 2, W + 2      # host-padded input height/width (114)
XFLAT = HP * WP            # 12996 flat padded-input elements per channel
OFLAT = H * WP             # 12768 flat padded-output elements per channel
BLK = 512                  # matmul free-dim block (= 1 PSUM bank of fp32)
NBLK = (OFLAT + BLK - 1) // BLK  # 25 blocks (24 full + 1 of 480)
XBUF = XFLAT + 8           # SBUF image stride (tap reads run to XFLAT+2)
GS = 8                     # out-DMA grouping: 8 blocks per transfer

# matmul dtype for legacy fp16 variants.
MM_DTYPE = mybir.dt.float32r

_cache = {}

# Variant switch:
#   "dr<k>" (k in 0,2,4) - fp8 e4m3 DoubleRow. k taps with single-fp8
#        weights, 9-k taps with hi+lo split weights. Host-measured rel err:
#        dr0 ~1e-3 / dr2 ~1.2e-2 / dr4 ~1.6e-2 (gate 2e-2).
#        Matmuls per block: (18-k)/2.
#   "pack5_fp16" - fp16, 5 K=128 MMs per block: 3 vertical tap-pairs from
#        the [x; x+WP] tile plus 2 MMs from a second [x+2WP; x+2WP+1] tile
#        covering row-2 taps ((2,0)+(2,1) paired, (2,2) with zero upper).
#        Drains alternate DVE/Act.
#   "pack6k128_fp16" - previous fp16 baseline (6 K=128 MMs per block).
#   "pack6", "pack6_bf16", "pack6k128_bf16", "wsplit9_bf16", "wsplit9_ldw",
#   "mm_only", "dma_only" - legacy/probe variants (see git history).
VARIANT = "pack5gs4"

# fp16 output for fp16/fp8 variants (host upcasts); halves out-DMA traffic.
OUT_FP16 = True


def _dr_k(variant):
    return int(variant[2:]) if variant.startswith("dr") else None


# DoubleRow plane-pair plans. Each MM is ((tap0, var0), (tap1, var1));
# var 0 = fp8(w), var 1 = fp8(w - fp8(w)). Taps not appearing with var 1
# use single-fp8 weights. HW CONSTRAINT: the within-partition stride
# between the two planes (s1 = off(tap1) - off(tap0)) must be EVEN --
# odd strides hard-fault the PE. Tap offsets dh*114+dw have parity dw%2,
# so every pair stays within one column-parity class (s1 in {2,114,228}).
_DR_PLANS = {
    # k=4: singles {0,2,1,4}, splits {3,5,6,8} (even) + {7} (odd).
    4: [
        ((0, 0), (2, 0)),
        ((1, 0), (7, 0)),
        ((4, 0), (7, 1)),
        ((3, 0), (5, 0)),
        ((3, 1), (5, 1)),
        ((6, 0), (8, 0)),
        ((6, 1), (8, 1)),
    ],
    # k=2: singles {1,4}.
    2: [
        ((1, 0), (7, 0)),
        ((4, 0), (7, 1)),
        ((0, 0), (2, 0)),
        ((0, 1), (2, 1)),
        ((3, 0), (5, 0)),
        ((3, 1), (5, 1)),
        ((6, 0), (8, 0)),
        ((6, 1), (8, 1)),
    ],
    # k=0: all taps split.
    0: [
        ((1, 0), (4, 0)),
        ((1, 1), (7, 0)),
        ((4, 1), (7, 1)),
        ((0, 0), (2, 0)),
        ((0, 1), (2, 1)),
        ((3, 0), (5, 0)),
        ((3, 1), (5, 1)),
        ((6, 0), (8, 0)),
        ((6, 1), (8, 1)),
    ],
}


def _dr_plan(k):
    plan = _DR_PLANS[k]
    for a, b in plan:
        s1 = _tap_off(b[0]) - _tap_off(a[0])
        assert s1 > 0 and s1 % 2 == 0, (a, b, s1)
    return plan


def _tap_off(t):
    dh, dw = divmod(t, KW)
    return dh * WP + dw


def _build(repeat=1):
    """Build + compile the per-core Bass program (cached per process).

    repeat>1 runs the whole per-core conv `repeat` times back-to-back inside
    one NEFF (idempotent) -- used by test.py to measure steady-state device
    time net of dispatch overhead.
    """
    key = ("nc", repeat, VARIANT)
    if key in _cache:
        return _cache[key]
    variant = VARIANT
    k = _dr_k(variant)
    if k is not None:
        nc = _build_dr(k, repeat)
        _cache[key] = nc
        return nc
    if variant.startswith("pack5") or variant == "p5mm":
        mode = ""
        gsv = GS
        if variant in ("pack5gs4", "pack5s", "pack5b3", "pack5sb3", "p5mm"):
            gsv = 4
        if variant == "pack5gs5":
            gsv = 5
        blkn = BLK
        if variant == "pack5n1024":
            gsv, blkn = 2, 1024
        if variant == "p5mm":
            mode = "mm"
        elif variant == "pack5s":
            mode = "s"
        elif variant == "pack5b3":
            mode = "b3"
        elif variant == "pack5sb3":
            mode = "sb3"
        nc = _build_pack5(repeat, gsv=gsv, mode=mode, blkn=blkn)
        _cache[key] = nc
        return nc

    nc = bacc.Bacc("TRN2", target_bir_lowering=False, debug=False)
    if variant.endswith("fp16") or variant.startswith("p6"):
        mm_dt = mybir.dt.float16
    elif variant.endswith("bf16") or variant in ("mm_only", "wsplit9_ldw"):
        mm_dt = mybir.dt.bfloat16
    else:
        mm_dt = MM_DTYPE
    nslot = 9 if variant in ("wsplit9_bf16", "mm_only", "wsplit9_ldw") else 6
    assert variant in (
        "pack6",
        "pack6_bf16",
        "pack6k128_bf16",
        "pack6k128_fp16",
        "pack6k128i_fp16",
        "p6mm",
        "p6mm1x",
        "p6md",
        "p6md1",
        "p6gs4",
        "p6gs2",
        "p6gs4b8",
        "p6gs16",
        "p6tm",
        "p6tm2",
        "wsplit9_bf16",
        "wsplit9_ldw",
        "mm_only",
        "dma_only",
    ), variant
    # x arrives pre-doubled from the host: per image a [128, XBUF] block whose
    # partitions 0-63 hold the padded image (rows 0-113) and partitions 64-127
    # the same image shifted one row (pack6*) or repeated (wsplit9), so one
    # full-width DMA loads both copies.
    x_ap = nc.dram_tensor(
        "x", [BPC, 2 * C, XBUF], mm_dt, kind="ExternalInput"
    ).ap()
    w_ap = nc.dram_tensor(
        "w", [2 * C, nslot * OC], mm_dt, kind="ExternalInput"
    ).ap()
    out_dt = (
        mybir.dt.float16
        if (OUT_FP16 and mm_dt == mybir.dt.float16)
        else mybir.dt.float32
    )
    out_ap = nc.dram_tensor(
        "out", [BPC, OC, H, WP], out_dt, kind="ExternalOutput"
    ).ap()
    assert not (variant == "p6mm" and repeat == 1) or True

    with tile.TileContext(nc) as tc:
        with (
            tc.tile_pool(name="xpool", bufs=3) as xpool,
            tc.tile_pool(name="wpool", bufs=1) as wpool,
            tc.tile_pool(name="opool", bufs=4) as opool,
            tc.tile_pool(name="psum", bufs=8, space="PSUM") as pspool,
        ):
            wt = wpool.tile([2 * C, nslot * OC], mm_dt)
            nc.sync.dma_start(wt[:], w_ap[:])

            def conv_pass_interleaved():
                # Two blocks' accumulation chains interleaved on the PE so
                # consecutive instructions hit different PSUM banks -- back-
                # to-back dependent matmuls otherwise stall ~100ns each on
                # the PE's SBUF-access pipeline refill.
                for im in range(BPC):
                    xt = xpool.tile([2 * C, XBUF], mm_dt)
                    nc.sync.dma_start(xt[:], x_ap[im])
                    o_im = out_ap[im].rearrange("o h w -> o (h w)")

                    ot = None
                    for pb in range(0, NBLK, 2):
                        blks = [b for b in (pb, pb + 1) if b < NBLK]
                        if pb % GS == 0:
                            ot = opool.tile([OC, GS * BLK], out_dt)
                            g0 = pb * BLK
                        pss = [
                            pspool.tile(
                                [OC, BLK], mybir.dt.float32, name="ps", tag="ps"
                            )
                            for _ in blks
                        ]
                        ns = [min(BLK, OFLAT - b * BLK) for b in blks]
                        for m in range(6):
                            for bi, blk in enumerate(blks):
                                j0 = blk * BLK
                                n = ns[bi]
                                if m < 3:
                                    off = j0 + m
                                    w_sl = wt[:, m * OC : (m + 1) * OC]
                                else:
                                    off = j0 + WP + (m - 3)
                                    w_sl = wt[:, m * OC : (m + 1) * OC]
                                nc.tensor.matmul(
                                    pss[bi][:, :n],
                                    lhsT=w_sl,
                                    rhs=xt[:, off : off + n],
                                    start=(m == 0),
                                    stop=(m == 5),
                                )
                        for bi, blk in enumerate(blks):
                            g = blk % GS
                            dst = ot[:, g * BLK : g * BLK + ns[bi]]
                            if bi == 0:
                                nc.vector.tensor_copy(dst, pss[bi][:, : ns[bi]])
                            else:
                                nc.scalar.copy(dst, pss[bi][:, : ns[bi]])
                        lastblk = blks[-1]
                        if lastblk % GS == GS - 1 or lastblk == NBLK - 1:
                            gn = lastblk * BLK + ns[-1] - g0
                            nc.sync.dma_start(o_im[:, g0 : g0 + gn], ot[:, :gn])

            GSV = {
                "p6gs4": 4,
                "p6gs2": 2,
                "p6gs4b8": 4,
                "p6gs16": 16,
                "p6tm": 4,
                "p6tm2": 2,
            }.get(variant, GS)

            def conv_pass_tapmajor(tg):
                # Tap-major over groups of `tg` blocks: one weight load per
                # tap per group, the other tg-1 matmuls reuse the loaded
                # weights (ldweights=False; PE executes in FIFO order).
                # Out-DMA group == tap group. Drains on DVE only.
                for im in range(BPC):
                    xt = xpool.tile([2 * C, XBUF], mm_dt)
                    nc.sync.dma_start(xt[:], x_ap[im])
                    o_im = out_ap[im].rearrange("o h w -> o (h w)")
                    for g0blk in range(0, NBLK, tg):
                        blks = list(range(g0blk, min(g0blk + tg, NBLK)))
                        g0 = g0blk * BLK
                        ot = opool.tile([OC, tg * BLK], out_dt)
                        pss = [
                            pspool.tile(
                                [OC, BLK], mybir.dt.float32, name="ps", tag="ps"
                            )
                            for _ in blks
                        ]
                        ns = [min(BLK, OFLAT - b * BLK) for b in blks]
                        for m in range(6):
                            for bi, blk in enumerate(blks):
                                j0 = blk * BLK
                                off = j0 + m if m < 3 else j0 + WP + (m - 3)
                                mm = nc.tensor.matmul(
                                    pss[bi][:, : ns[bi]],
                                    lhsT=wt[:, m * OC : (m + 1) * OC],
                                    rhs=xt[:, off : off + ns[bi]],
                                    start=(m == 0),
                                    stop=(m == 5),
                                )
                                if bi > 0:
                                    mm.ldweights = False
                        for bi, blk in enumerate(blks):
                            dst = ot[:, bi * BLK : bi * BLK + ns[bi]]
                            nc.vector.tensor_copy(dst, pss[bi][:, : ns[bi]])
                        gn = blks[-1] * BLK + ns[-1] - g0
                        nc.sync.dma_start(o_im[:, g0 : g0 + gn], ot[:, :gn])

            def conv_pass():
                if variant == "pack6k128i_fp16":
                    conv_pass_interleaved()
                    return
                if variant in ("p6tm", "p6tm2"):
                    conv_pass_tapmajor(GSV)
                    return
                xt_shared = None
                for im in range(BPC):
                    if variant == "p6mm1x":
                        if xt_shared is None:
                            xt_shared = xpool.tile([2 * C, XBUF], mm_dt)
                            nc.sync.dma_start(xt_shared[:], x_ap[0])
                        xt = xt_shared
                    else:
                        xt = xpool.tile([2 * C, XBUF], mm_dt)
                        nc.sync.dma_start(xt[:], x_ap[im])
                    o_im = out_ap[im].rearrange("o h w -> o (h w)")

                    ot = None
                    for blk in range(NBLK):
                        j0 = blk * BLK
                        n = min(BLK, OFLAT - j0)
                        g = blk % GSV
                        if g == 0:
                            ot = opool.tile(
                                [OC, GSV * BLK],
                                out_dt,
                                bufs=8 if variant == "p6gs4b8" else None,
                            )
                            g0 = j0
                        if variant == "dma_only":
                            nc.vector.tensor_copy(
                                ot[:, g * BLK : g * BLK + n], xt[:OC, j0 : j0 + n]
                            )
                        elif variant in ("wsplit9_bf16", "mm_only"):
                            ps = pspool.tile([OC, BLK], mybir.dt.float32)
                            for t in range(KH * KW):
                                dh, dw = divmod(t, KW)
                                off = j0 + dh * WP + dw
                                nc.tensor.matmul(
                                    ps[:, :n],
                                    lhsT=wt[:, t * OC : (t + 1) * OC],
                                    rhs=xt[:, off : off + n],
                                    start=(t == 0),
                                    stop=(t == KH * KW - 1),
                                )
                            if variant == "mm_only":
                                continue
                            nc.vector.tensor_copy(
                                ot[:, g * BLK : g * BLK + n], ps[:, :n]
                            )
                        else:
                            ps = pspool.tile([OC, BLK], mybir.dt.float32)
                            k128 = variant.startswith(("pack6k128", "p6"))
                            for d in range(3):
                                nc.tensor.matmul(
                                    ps[:, :n],
                                    lhsT=wt[:, d * OC : (d + 1) * OC],
                                    rhs=xt[:, j0 + d : j0 + d + n],
                                    start=(d == 0),
                                    stop=False,
                                )
                            for d in range(3):
                                if k128:
                                    nc.tensor.matmul(
                                        ps[:, :n],
                                        lhsT=wt[:, (3 + d) * OC : (4 + d) * OC],
                                        rhs=xt[:, j0 + WP + d : j0 + WP + d + n],
                                        start=False,
                                        stop=(d == 2),
                                    )
                                else:
                                    nc.tensor.matmul(
                                        ps[:, :n],
                                        lhsT=wt[:C, (3 + d) * OC : (4 + d) * OC],
                                        rhs=xt[
                                            :C,
                                            j0 + 2 * WP + d : j0 + 2 * WP + d + n,
                                        ],
                                        start=False,
                                        stop=(d == 2),
                                    )
                            if variant in ("p6mm", "p6mm1x"):
                                continue
                            dst = ot[:, g * BLK : g * BLK + n]
                            if variant == "p6md" and blk % 2:
                                nc.scalar.copy(dst, ps[:, :n])
                            else:
                                nc.vector.tensor_copy(dst, ps[:, :n])
                            if variant in ("p6md", "p6md1"):
                                continue
                        if g == GSV - 1 or blk == NBLK - 1:
                            gn = j0 + n - g0
                            nc.sync.dma_start(
                                o_im[:, g0 : g0 + gn], ot[:, :gn]
                            )

            if repeat == 1:
                conv_pass()
            else:
                with tc.For_i(0, repeat, 1):
                    conv_pass()

    nc.compile()
    _cache[key] = nc
    return nc


OBUF = OFLAT + 4  # H-tile (row-2) columns per partition for pack5


def _build_pack5(repeat=1, gsv=GS, mode="", blkn=BLK):
    """fp16 5-MM variant: 3 vertical tap-pair MMs from [x; x+WP] plus 2 MMs
    from the appended [x+2WP; x+2WP+1] columns covering row-2 taps."""
    fp16 = mybir.dt.float16
    out_dt = mybir.dt.float16 if OUT_FP16 else mybir.dt.float32
    XW = XBUF + OBUF

    nc = bacc.Bacc("TRN2", target_bir_lowering=False, debug=False)
    xw_dram = XBUF if "s" in mode else XW
    x_ap = nc.dram_tensor(
        "x", [BPC, 2 * C, xw_dram], fp16, kind="ExternalInput"
    ).ap()
    w_ap = nc.dram_tensor("w", [2 * C, 5 * OC], fp16, kind="ExternalInput").ap()
    out_ap = nc.dram_tensor(
        "out", [BPC, OC, H, WP], out_dt, kind="ExternalOutput"
    ).ap()

    with tile.TileContext(nc) as tc:
        with (
            tc.tile_pool(name="xpool", bufs=3 if "b3" in mode else 2) as xpool,
            tc.tile_pool(name="wpool", bufs=1) as wpool,
            tc.tile_pool(name="opool", bufs=4) as opool,
            tc.tile_pool(name="psum", bufs=8, space="PSUM") as pspool,
        ):
            wt = wpool.tile([2 * C, 5 * OC], fp16)
            nc.sync.dma_start(wt[:], w_ap[:])

            nblkn = (OFLAT + blkn - 1) // blkn

            def conv_pass():
                nh = OFLAT + 2
                for im in range(BPC):
                    xt = xpool.tile([2 * C, XW], fp16)
                    if "s" in mode:
                        # HBM carries only the [x; x+WP] columns; the row-2
                        # H-columns are built by two same-partition SBUF->SBUF
                        # column-shift copies (halves HBM x traffic).
                        nc.sync.dma_start(xt[:, :XBUF], x_ap[im])
                        nc.sync.dma_start(
                            xt[:C, XBUF : XBUF + nh],
                            xt[:C, 2 * WP : 2 * WP + nh],
                        )
                        nc.sync.dma_start(
                            xt[C:, XBUF : XBUF + nh],
                            xt[C:, WP + 1 : WP + 1 + nh],
                        )
                    else:
                        nc.sync.dma_start(xt[:], x_ap[im])
                    o_im = out_ap[im].rearrange("o h w -> o (h w)")

                    ot = None
                    for blk in range(nblkn):
                        j0 = blk * blkn
                        n = min(blkn, OFLAT - j0)
                        g = blk % gsv
                        if g == 0:
                            ot = opool.tile([OC, gsv * blkn], out_dt)
                            g0 = j0
                        ps = pspool.tile(
                            [OC, blkn],
                            mybir.dt.float32,
                            bufs=8 if blkn <= BLK else 4,
                        )
                        for d in range(3):
                            nc.tensor.matmul(
                                ps[:, :n],
                                lhsT=wt[:, d * OC : (d + 1) * OC],
                                rhs=xt[:, j0 + d : j0 + d + n],
                                start=(d == 0),
                                stop=False,
                            )
                        nc.tensor.matmul(
                            ps[:, :n],
                            lhsT=wt[:, 3 * OC : 4 * OC],
                            rhs=xt[:, XBUF + j0 : XBUF + j0 + n],
                            start=False,
                            stop=False,
                        )
                        nc.tensor.matmul(
                            ps[:, :n],
                            lhsT=wt[:, 4 * OC : 5 * OC],
                            rhs=xt[:, XBUF + j0 + 2 : XBUF + j0 + 2 + n],
                            start=False,
                            stop=True,
                        )
                        if "mm" in mode:
                            continue
                        dst = ot[:, g * blkn : g * blkn + n]
                        nc.vector.tensor_copy(dst, ps[:, :n])
                        if g == gsv - 1 or blk == nblkn - 1:
                            gn = j0 + n - g0
                            nc.sync.dma_start(o_im[:, g0 : g0 + gn], ot[:, :gn])

            if repeat == 1:
                conv_pass()
            else:
                with tc.For_i(0, repeat, 1):
                    conv_pass()

    nc.compile()
    return nc


def _prep_inputs_pack5(x, weight, v_only=False):
    xp = np.zeros((B, C, HP, WP), dtype=np.float32)
    xp[:, :, 1 : H + 1, 1 : W + 1] = x
    flat = xp.reshape(B, C, XFLAT).astype(np.float16)
    XW = XBUF if v_only else XBUF + OBUF
    xprep = np.zeros((B, 2 * C, XW), dtype=np.float16)
    xprep[:, :C, :XFLAT] = flat
    xprep[:, C:, : XFLAT - WP] = flat[:, :, WP:]
    if not v_only:
        nh = XFLAT - 2 * WP  # valid columns from x+2WP
        xprep[:, :C, XBUF : XBUF + nh] = flat[:, :, 2 * WP :]
        xprep[:, C:, XBUF : XBUF + nh - 1] = flat[:, :, 2 * WP + 1 :]

    w4 = weight[0].astype(np.float16)  # (out_c, in_c, kh, kw)
    wp = np.zeros((2 * C, 5, OC), dtype=np.float16)
    for d in range(KW):
        wp[:C, d] = w4[:, :, 0, d].T
        wp[C:, d] = w4[:, :, 1, d].T
    wp[:C, 3] = w4[:, :, 2, 0].T
    wp[C:, 3] = w4[:, :, 2, 1].T
    wp[:C, 4] = w4[:, :, 2, 2].T
    w_prep = np.ascontiguousarray(wp.reshape(2 * C, 5 * OC))
    return [
        {"x": xprep[c * BPC : (c + 1) * BPC], "w": w_prep} for c in range(NCORES)
    ]


def _build_dr(k, repeat=1):
    """fp8 e4m3 DoubleRow variant: see module docstring."""
    fp8 = mybir.dt.float8e4
    plan = _dr_plan(k)
    nmm = len(plan)
    out_dt = mybir.dt.float16 if OUT_FP16 else mybir.dt.float32

    nc = bacc.Bacc("TRN2", target_bir_lowering=False, debug=False)
    # Partitions 0-63: fp8(x) padded flat; 64-127: fp8 residual of the same.
    x_ap = nc.dram_tensor("x", [BPC, 2 * C, XBUF], fp8, kind="ExternalInput").ap()
    w_ap = nc.dram_tensor(
        "w", [2 * C, nmm * 2 * OC], fp8, kind="ExternalInput"
    ).ap()
    out_ap = nc.dram_tensor(
        "out", [BPC, OC, H, WP], out_dt, kind="ExternalOutput"
    ).ap()

    with tile.TileContext(nc) as tc:
        with (
            tc.tile_pool(name="xpool", bufs=3) as xpool,
            tc.tile_pool(name="wpool", bufs=1) as wpool,
            tc.tile_pool(name="opool", bufs=4) as opool,
            tc.tile_pool(name="psum", bufs=8, space="PSUM") as pspool,
        ):
            wt = wpool.tile([2 * C, nmm, 2, OC], fp8)
            nc.sync.dma_start(
                wt[:], w_ap[:].rearrange("p (m two o) -> p m two o", m=nmm, two=2)
            )

            def conv_pass():
                for im in range(BPC):
                    xt = xpool.tile([2 * C, XBUF], fp8)
                    nc.sync.dma_start(xt[:], x_ap[im])
                    xfull = xt[:]
                    xtensor, xoff, xpstride = (
                        xfull.tensor,
                        xfull.offset,
                        xfull.ap[0][0],
                    )
                    o_im = out_ap[im].rearrange("o h w -> o (h w)")

                    ot = None
                    for blk in range(NBLK):
                        j0 = blk * BLK
                        n = min(BLK, OFLAT - j0)
                        g = blk % GS
                        if g == 0:
                            ot = opool.tile([OC, GS * BLK], out_dt)
                            g0 = j0
                        ps = pspool.tile([OC, BLK], mybir.dt.float32)
                        for m, (a, b) in enumerate(plan):
                            o0 = _tap_off(a[0])
                            s1 = _tap_off(b[0]) - o0
                            rhs = bass.AP(
                                xtensor,
                                xoff + j0 + o0,
                                [[xpstride, 2 * C], [s1, 2], [1, n]],
                            )
                            nc.tensor.matmul(
                                ps[:, :n],
                                lhsT=wt[:, m],
                                rhs=rhs,
                                start=(m == 0),
                                stop=(m == nmm - 1),
                                perf_mode=mybir.MatmulPerfMode.DoubleRow,
                            )
                        # PSUM -> SBUF fp16 drain, alternating DVE / Act so
                        # neither engine becomes the bottleneck.
                        dst = ot[:, g * BLK : g * BLK + n]
                        if blk % 2 == 0:
                            nc.vector.tensor_copy(dst, ps[:, :n])
                        else:
                            nc.scalar.copy(dst, ps[:, :n])
                        if g == GS - 1 or blk == NBLK - 1:
                            gn = j0 + n - g0
                            nc.sync.dma_start(o_im[:, g0 : g0 + gn], ot[:, :gn])

            if repeat == 1:
                conv_pass()
            else:
                with tc.For_i(0, repeat, 1):
                    conv_pass()

    nc.compile()
    return nc


def run_on_device(nc, in_maps):
    """Single-exec jitted runner with device-resident inputs; returns a
    callable for repeated timing plus the output fetcher."""
    from jax.sharding import Mesh, NamedSharding, PartitionSpec
    from jax.experimental.shard_map import shard_map
    import jax

    from concourse.bass2jax import (
        _bass_exec_p,
        install_neuronx_cc_hook,
        partition_id_tensor,
    )

    install_neuronx_cc_hook()

    partition_name = nc.partition_id_tensor.name if nc.partition_id_tensor else None
    in_names, out_names, out_avals = [], [], []
    for alloc in nc.m.functions[0].allocations:
        if not isinstance(alloc, mybir.MemoryLocationSet):
            continue
        name = alloc.memorylocations[0].name
        if alloc.kind == "ExternalInput":
            if name != partition_name:
                in_names.append(name)
        elif alloc.kind == "ExternalOutput":
            out_names.append(name)
            out_avals.append(
                jax.core.ShapedArray(
                    tuple(alloc.tensor_shape), mybir.dt.np(alloc.dtype)
                )
            )
    n_params = len(in_names)
    all_in_names = list(in_names) + list(out_names)
    if partition_name is not None:
        all_in_names.append(partition_name)
    all_in_names = tuple(all_in_names)

    def body(*args):
        operands = list(args)
        if partition_name is not None:
            operands.append(partition_id_tensor())
        return tuple(
            _bass_exec_p.bind(
                *operands,
                out_avals=tuple(out_avals),
                in_names=all_in_names,
                out_names=tuple(out_names),
                lowering_input_output_aliases=(),
                sim_require_finite=True,
                sim_require_nnan=True,
                nc=nc,
            )
        )

    n_cores = len(in_maps)
    devices = jax.devices()[:n_cores]
    mesh = Mesh(np.asarray(devices), ("core",))
    nspecs = n_params + len(out_names)
    sharded = jax.jit(
        shard_map(
            body,
            mesh=mesh,
            in_specs=(PartitionSpec("core"),) * nspecs,
            out_specs=(PartitionSpec("core"),) * len(out_names),
            check_rep=False,
        )
    )
    concat_in = [
        np.concatenate([np.asarray(in_maps[c][nm]) for c in range(n_cores)], axis=0)
        for nm in in_names
    ]
    concat_zeros = [
        np.zeros((n_cores * a.shape[0], *a.shape[1:]), a.dtype) for a in out_avals
    ]
    sharding = NamedSharding(mesh, PartitionSpec("core"))
    dev_in = [jax.device_put(a, sharding) for a in concat_in]
    dev_zeros = [jax.device_put(a, sharding) for a in concat_zeros]

    def run():
        return sharded(*dev_in, *dev_zeros)

    return run, out_names, out_avals


def _prep_inputs_dr(x, weight, k):
    import ml_dtypes

    FP8 = ml_dtypes.float8_e4m3
    plan = _dr_plan(k)
    nmm = len(plan)

    xp = np.zeros((B, C, HP, WP), dtype=np.float32)
    xp[:, :, 1 : H + 1, 1 : W + 1] = x
    flat = xp.reshape(B, C, XFLAT)
    hi = flat.astype(FP8)
    lo = (flat - hi.astype(np.float32)).astype(FP8)
    xprep = np.zeros((B, 2 * C, XBUF), dtype=FP8)
    xprep[:, :C, :XFLAT] = hi
    xprep[:, C:, :XFLAT] = lo

    w4 = weight[0]  # (out_c, in_c, kh, kw)
    w_hi = w4.astype(FP8)
    w_lo = (w4.astype(np.float32) - w_hi.astype(np.float32)).astype(FP8)
    wvar = (w_hi, w_lo)
    wp = np.zeros((2 * C, nmm, 2, OC), dtype=FP8)
    for m, pair in enumerate(plan):
        for i, (t, v) in enumerate(pair):
            kh, kw = divmod(t, KW)
            wmat = wvar[v][:, :, kh, kw].T  # (in_c, out_c)
            wp[:C, m, i] = wmat
            wp[C:, m, i] = wmat
    w_prep = np.ascontiguousarray(wp.reshape(2 * C, nmm * 2 * OC))
    return [
        {"x": xprep[c * BPC : (c + 1) * BPC], "w": w_prep} for c in range(NCORES)
    ]


def _prep_inputs(x, weight):
    """Host-side shard + layout prep. Returns per-core input maps."""
    import ml_dtypes

    variant = VARIANT
    k = _dr_k(variant)
    if k is not None:
        return _prep_inputs_dr(x, weight, k)
    if variant.startswith("pack5") or variant == "p5mm":
        return _prep_inputs_pack5(
            x, weight, v_only=(variant in ("pack5s", "pack5sb3"))
        )

    if variant.endswith("fp16"):
        host_dt = np.float16
    elif variant.endswith("bf16") or variant in ("mm_only", "wsplit9_ldw"):
        host_dt = ml_dtypes.bfloat16
    else:
        host_dt = np.float32

    xp = np.zeros((B, C, HP, WP), dtype=np.float32)
    xp[:, :, 1 : H + 1, 1 : W + 1] = x
    flat = xp.reshape(B, C, XFLAT).astype(host_dt)
    xprep = np.zeros((B, 2 * C, XBUF), dtype=host_dt)
    xprep[:, :C, :XFLAT] = flat
    if variant in ("wsplit9_bf16", "mm_only", "wsplit9_ldw"):
        xprep[:, C:, :XFLAT] = flat
    else:
        xprep[:, C:, : XFLAT - WP] = flat[:, :, WP:]

    w4 = weight[0]  # (out_c, in_c, kh, kw)
    if variant in ("wsplit9_bf16", "mm_only", "wsplit9_ldw"):
        w_hi = w4.astype(ml_dtypes.bfloat16)
        w_lo = (w4.astype(np.float32) - w_hi.astype(np.float32)).astype(
            ml_dtypes.bfloat16
        )
        wp = np.zeros((2 * C, KH * KW, OC), dtype=host_dt)
        for t in range(KH * KW):
            kh, kw = divmod(t, KW)
            wp[:C, t] = w_hi[:, :, kh, kw].T
            wp[C:, t] = w_lo[:, :, kh, kw].T
        w_prep = np.ascontiguousarray(wp.reshape(2 * C, KH * KW * OC))
    else:
        wp = np.zeros((2 * C, 6, OC), dtype=host_dt)
        for d in range(KW):
            wp[:C, d] = w4[:, :, 0, d].T.astype(host_dt)
            wp[C:, d] = w4[:, :, 1, d].T.astype(host_dt)
            if variant.startswith("pack6k128"):
                wp[C:, 3 + d] = w4[:, :, 2, d].T.astype(host_dt)
            else:
                wp[:C, 3 + d] = w4[:, :, 2, d].T.astype(host_dt)
        w_prep = np.ascontiguousarray(wp.reshape(2 * C, 6 * OC))
    return [
        {"x": xprep[c * BPC : (c + 1) * BPC], "w": w_prep} for c in range(NCORES)
    ]


def kernel(x, weight):
    x = np.asarray(x, dtype=np.float32)
    weight = np.asarray(weight, dtype=np.float32)
    nc = _build()
    in_maps = _prep_inputs(x, weight)
    # Retry on transient device failures (a crashed prior process can leave
    # the first subsequent execution returning UNAVAILABLE or garbage) and
    # validate the output is finite before returning.
    last_exc = None
    for _attempt in range(3):
        try:
            res = run_bass_kernel_spmd(nc, in_maps, list(range(NCORES)))
            out = np.concatenate(
                [res.results[c]["out"] for c in range(NCORES)], axis=0
            )
        except Exception as exc:  # noqa: BLE001 - retry any runtime failure
            last_exc = exc
            continue
        out = np.ascontiguousarray(out[:, :, :, :W].astype(np.float32))
        if np.isfinite(out).all():
            return out
    if last_exc is not None:
        raise last_exc
    return out
